# revision 1
# baseline (speedup 1.0000x reference)
"""Trainium2 Bass kernel for the sum-product "knowledge layer" network.

Computation (see problem reference):
  h0 = encode(x): 8194-row table [-inf, 0, pos0, neg0, pos1, neg1, ...]
       with pos = x (log-probs), neg = log(1 - exp(x)), per batch column.
  4 alternating layers, each: gather rows by ptrs, then segment-reduce over
  contiguous fanin groups (fanin 4 sum-of-logs "product" layers, fanin 2
  logsumexp "sum" layers).

Strategy (pure batch data-parallelism, 8 NeuronCores):
  - Shard the 512 batch columns 8 ways -> 64 columns per core.
  - Per core every tensor lives in DRAM as [rows, 64] fp32; one row = 256B.
  - Gathers use the SWDGE dma_gather instruction: int16 index list in SBUF,
    each index pulls one 256B row from the DRAM table; index list position j
    lands at SBUF partition j%128, free slot j//128.
  - Host pre-permutes each layer's ptrs so that the edges of output group g
    land on partition g//C (C = n_out/128) at free slots fanin*(g%C)+k.
    Segment reduction then becomes strided free-dim vector ops, and the
    layer output [128, C, 64] DMAs back to DRAM in natural row order
    (partition p holds rows p*C .. p*C+C-1, fully contiguous per partition).
  - Sum layers: logsumexp(a,b) = max + softplus(min - max) on DVE + ACT.
"""

import numpy as np

P = 128
B = 64  # batch columns per core
NCORES = 8
N_VARS = 4096
BATCH = 512
TAB0 = 2 * N_VARS + 2  # 8194
OUT_SIZES = [16384, 8192, 4096, 2048]
FANINS = [4, 2, 4, 2]
CHUNK = 8192  # gather indices per dma_gather instruction


def layer_specs(out_sizes, fanins, tab0):
    specs = []
    prev = tab0
    for n_out, f in zip(out_sizes, fanins):
        specs.append({"f": f, "n_in": prev, "n_out": n_out, "n_edges": n_out * f})
        prev = n_out
    return specs


def reorder_wrap(ptrs, f, n_out):
    """Permute edge pointers into dma_gather order and wrap into the int16
    [128, n_edges//16] SBUF layout (position j -> [j%16, j//16], replicated
    across the 8 gpsimd cores' 16-partition groups)."""
    C = n_out // P
    n_edges = n_out * f
    j = np.arange(n_edges)
    p = j % P
    slot = j // P
    c = slot // f
    k = slot % f
    g = p * C + c
    src = np.asarray(ptrs).astype(np.int64)[g * f + k]
    assert src.max() < 2**15 and src.min() >= 0
    src = src.astype(np.int16)
    return np.ascontiguousarray(np.tile(src.reshape(-1, 16).T, (8, 1)))


def build_nc(n_vars=N_VARS, out_sizes=OUT_SIZES, fanins=FANINS, chunk=CHUNK):
    import concourse.bacc as bacc
    import concourse.mybir as mybir
    import concourse.tile as tile

    f32 = mybir.dt.float32
    i16 = mybir.dt.int16
    Alu = mybir.AluOpType
    Act = mybir.ActivationFunctionType

    tab0 = 2 * n_vars + 2
    specs = layer_specs(out_sizes, fanins, tab0)
    S_ENC = n_vars // P  # encode slots per partition

    nc = bacc.Bacc("TRN2", target_bir_lowering=False, debug=False)
    x = nc.dram_tensor("x", [P, S_ENC * B], f32, kind="ExternalInput")
    idx_in = [
        nc.dram_tensor(f"idx{l}", [P, s["n_edges"] // 16], i16, kind="ExternalInput")
        for l, s in enumerate(specs)
    ]
    out = nc.dram_tensor("out", [out_sizes[-1], B], f32, kind="ExternalOutput")

    with tile.TileContext(nc) as tc:
        with (
            tc.tile_pool(name="dram", bufs=1, space="DRAM") as dpool,
            tc.tile_pool(name="sb", bufs=4) as gp,
            tc.tile_pool(name="hb", bufs=3) as hp,
            tc.tile_pool(name="tmp", bufs=2) as tp,
            tc.tile_pool(name="ix", bufs=1) as ixp,
        ):
            tables = [
                dpool.tile([s["n_in"], B], f32, name=f"t{l}", tag=f"t{l}")
                for l, s in enumerate(specs)
            ]

            # --- index list loads ---
            ix_t = []
            for l, s in enumerate(specs):
                t = ixp.tile([P, s["n_edges"] // 16], i16, tag=f"ix{l}")
                nc.sync.dma_start(t[:], idx_in[l][:])
                ix_t.append(t)

            # --- encode: pos rows at 2+2i, neg rows at 3+2i, zeros at row 1.
            # Partition p computes vars p*S_ENC .. p*S_ENC+S_ENC-1 so the
            # interleaved pos/neg store is one contiguous run per partition.
            iv = gp.tile([P, S_ENC, 2, B], f32, tag="g")
            nc.sync.dma_start(
                iv[:][:, :, 0, :], x[:].rearrange("p (s b) -> p s b", b=B)
            )
            et = hp.tile([P, S_ENC, B], f32, tag="h")
            nc.scalar.activation(et[:], iv[:][:, :, 0, :], Act.Exp)
            nc.scalar.activation(iv[:][:, :, 1, :], et[:], Act.Ln, scale=-1.0, bias=1.0)
            nc.sync.dma_start(
                tables[0][:][2:, :].rearrange("(p s k) b -> p (s k b)", p=P, k=2),
                iv[:].rearrange("p s k b -> p (s k b)"),
            )
            # rows 0 (-inf in the reference, never gathered) and 1 (zeros)
            z = ixp.tile([2, B], f32, tag="z")
            nc.vector.memset(z[:], 0.0)
            nc.sync.dma_start(tables[0][:][0:2, :], z[:])

            # --- gather + segment-reduce layers ---
            for l, s in enumerate(specs):
                f, n_out, n_edges = s["f"], s["n_out"], s["n_edges"]
                C = n_out // P
                ch = min(chunk if f == 4 else chunk // 2, n_edges)
                assert n_edges % ch == 0
                S = ch // P  # slots per chunk
                Csub = S // f  # groups per partition per chunk
                src_ap = tables[l][:]
                dst_full = (tables[l + 1][:] if l + 1 < len(specs) else out[:]).rearrange(
                    "(p C) b -> p C b", p=P
                )
                for ci in range(n_edges // ch):
                    g = gp.tile([P, S, B], f32, tag="g")
                    nc.gpsimd.dma_gather(
                        g[:],
                        src_ap,
                        ix_t[l][:, ci * (ch // 16) : (ci + 1) * (ch // 16)],
                        ch,
                        ch,
                        B,
                        single_packet=False,
                    )
                    v = g[:].rearrange("p (c k) b -> p c k b", k=f)
                    h = hp.tile([P, Csub, B], f32, tag="h")
                    if f == 4:
                        s01 = tp.tile([P, Csub, B], f32, tag="m")
                        s23 = tp.tile([P, Csub, B], f32, tag="n")
                        nc.vector.tensor_add(s01[:], v[:, :, 0, :], v[:, :, 1, :])
                        nc.vector.tensor_add(s23[:], v[:, :, 2, :], v[:, :, 3, :])
                        nc.vector.tensor_add(h[:], s01[:], s23[:])
                    else:
                        # logsumexp(a,b) = max + ln(1 + exp(min - max))
                        m = tp.tile([P, Csub, B], f32, tag="m")
                        mn = tp.tile([P, Csub, B], f32, tag="n")
                        d = tp.tile([P, Csub, B], f32, tag="d")
                        sp = tp.tile([P, Csub, B], f32, tag="sp")
                        nc.vector.tensor_tensor(
                            m[:], v[:, :, 0, :], v[:, :, 1, :], op=Alu.max
                        )
                        nc.vector.tensor_tensor(
                            mn[:], v[:, :, 0, :], v[:, :, 1, :], op=Alu.min
                        )
                        nc.vector.tensor_tensor(d[:], mn[:], m[:], op=Alu.subtract)
                        nc.scalar.activation(d[:], d[:], Act.Exp)
                        nc.scalar.activation(sp[:], d[:], Act.Ln, bias=1.0)
                        nc.vector.tensor_add(h[:], m[:], sp[:])
                    nc.sync.dma_start(
                        dst_full[:, ci * Csub : (ci + 1) * Csub, :], h[:]
                    )
    nc.compile()
    return nc


def host_prep(x, ptrs_list, seg_list, n_vars=N_VARS, out_sizes=OUT_SIZES, fanins=FANINS):
    """Host-side sharding + index preprocessing. Returns per-core input maps."""
    x = np.asarray(x, dtype=np.float32)
    specs = layer_specs(out_sizes, fanins, 2 * n_vars + 2)
    idx_maps = {}
    for l, s in enumerate(specs):
        seg = np.asarray(seg_list[l]).astype(np.int64)
        expected = np.repeat(np.arange(s["n_out"], dtype=np.int64), s["f"])
        assert np.array_equal(seg, expected), f"layer {l}: non-uniform segments"
        idx_maps[f"idx{l}"] = reorder_wrap(ptrs_list[l], s["f"], s["n_out"])

    batch = x.shape[1]
    bpc = batch // NCORES
    in_maps = []
    for i in range(NCORES):
        xs = x[:, i * bpc : (i + 1) * bpc]
        # partition p holds vars p*S_ENC .. p*S_ENC+S_ENC-1 (natural order)
        xv = np.ascontiguousarray(xs).reshape(P, -1)
        in_maps.append({"x": xv, **idx_maps})
    return in_maps


_CACHE = {}


def _get_nc():
    if "nc" not in _CACHE:
        _CACHE["nc"] = build_nc()
    return _CACHE["nc"]


def kernel(x, ptrs0, seg0, ptrs1, seg1, ptrs2, seg2, ptrs3, seg3):
    from concourse.bass_utils import run_bass_kernel_spmd

    nc = _get_nc()
    in_maps = host_prep(
        x, [ptrs0, ptrs1, ptrs2, ptrs3], [seg0, seg1, seg2, seg3]
    )
    res = run_bass_kernel_spmd(nc, in_maps, core_ids=list(range(NCORES)))
    outs = [r["out"] for r in res.results]
    return np.concatenate(outs, axis=1)



# revision 4
# speedup vs baseline: 1.4359x; 1.4359x over previous
"""Trainium2 Bass kernel for the sum-product "knowledge layer" network.

Computation (see problem reference):
  h0 = encode(x): 8194-row table [-inf, 0, pos0, neg0, pos1, neg1, ...]
       with pos = x (log-probs), neg = log(1 - exp(x)), per batch column.
  4 alternating layers, each: gather rows by ptrs, then segment-reduce over
  contiguous fanin groups (fanin 4 sum-of-logs "product" layers, fanin 2
  logsumexp "sum" layers).

Strategy (pure batch data-parallelism, 8 NeuronCores):
  - Shard the 512 batch columns 8 ways -> 64 columns per core.
  - Per core every tensor lives in DRAM as [rows, 64] fp32; one row = 256B.
  - Gathers use the SWDGE dma_gather instruction: int16 index list in SBUF,
    each index pulls one 256B row from the DRAM table; index list position j
    lands at SBUF partition j%128, free slot j//128.
  - DAG pruning (host side, per input set): working back from the 2048
    output rows, only rows actually referenced downstream are computed.
    Each layer's output table is compacted (relabeled); the relabeling is
    folded into the next layer's gather indices. This removes ~40% of all
    gather descriptors (the dominant cost) and shrinks the table stores.
  - Host pre-permutes each layer's edge list so that the edges of compacted
    output group g land on partition g//C (C = n_groups/128) at free slots
    fanin*(g%C)+k. Segment reduction then becomes strided free-dim vector
    ops, and the layer output [128, C, 64] DMAs back to DRAM in natural row
    order (partition p holds rows p*C .. p*C+C-1, contiguous per partition).
  - Sum layers: logsumexp(a,b) = a + softplus(b-a) where the argument range
    is f32-exp-safe (layer 1), else max + softplus(min-max) (layer 3).
"""

import numpy as np

P = 128
B = 64  # batch columns per core
NCORES = 8
N_VARS = 4096
BATCH = 512
TAB0 = 2 * N_VARS + 2  # 8194
OUT_SIZES = [16384, 8192, 4096, 2048]
FANINS = [4, 2, 4, 2]
CHUNK = 8192  # max gather indices per dma_gather instruction


def _pad_groups(n):
    return -(-n // P) * P


def plan(ptrs_list):
    """Prune the DAG bottom-up and compact each layer.

    Returns per-layer dicts with:
      f: fanin, n_groups: padded compacted group count,
      src: per-edge source-row ids (into the previous compacted table,
           length f*n_groups, grouped by output row, padding groups -> 0),
      n_src_rows: row count of the table this layer gathers from.
    """
    p0, p1, p2, p3 = [np.asarray(p).astype(np.int64) for p in ptrs_list]
    used3 = np.arange(OUT_SIZES[3], dtype=np.int64)  # all outputs needed
    e3 = p3.reshape(-1, 2)
    used2 = np.unique(e3)
    e2 = p2.reshape(-1, 4)[used2]
    used1 = np.unique(e2)
    e1 = p1.reshape(-1, 2)[used1]
    used0 = np.unique(e1)
    e0 = p0.reshape(-1, 4)[used0]

    def relabel(edges, used):
        return np.searchsorted(used, edges)

    layers = []
    prev_rows = TAB0
    for edges, used_src, n_groups_exact in (
        (e0, None, used0.size),
        (relabel(e1, used0), used0, used1.size),
        (relabel(e2, used1), used1, used2.size),
        (relabel(e3, used2), used2, OUT_SIZES[3]),
    ):
        f = edges.shape[1]
        n_pad = _pad_groups(n_groups_exact)
        src = np.zeros((n_pad, f), dtype=np.int64)
        src[:n_groups_exact] = edges
        layers.append(
            {"f": f, "n_groups": n_pad, "src": src.ravel(), "n_src_rows": prev_rows}
        )
        prev_rows = n_pad
    return layers


def reorder_wrap(src, f, n_groups):
    """Permute per-edge source ids into dma_gather order and wrap into the
    int16 [128, n_edges//16] SBUF layout (position j -> [j%16, j//16],
    replicated across the 8 gpsimd cores' 16-partition groups)."""
    C = n_groups // P
    n_edges = n_groups * f
    j = np.arange(n_edges)
    p = j % P
    slot = j // P
    c = slot // f
    k = slot % f
    g = p * C + c
    out = src[g * f + k]
    assert out.max() < 2**15 and out.min() >= 0
    out = out.astype(np.int16)
    return np.ascontiguousarray(np.tile(out.reshape(-1, 16).T, (8, 1)))


def _chunk_sizes(n_edges, f):
    """Split n_edges into dma_gather chunks: full CHUNK-sized plus one
    remainder, every chunk a multiple of f*128 edges."""
    sizes = []
    left = n_edges
    while left > 0:
        c = min(CHUNK, left)
        assert c % (f * P) == 0
        sizes.append(c)
        left -= c
    return sizes


def build_nc(counts):
    """counts: (n0p, n1p, n2p) padded compacted group counts for layers 0-2."""
    import concourse.bacc as bacc
    import concourse.mybir as mybir
    import concourse.tile as tile

    f32 = mybir.dt.float32
    i16 = mybir.dt.int16
    Alu = mybir.AluOpType
    Act = mybir.ActivationFunctionType

    n0p, n1p, n2p = counts
    specs = [
        {"f": 4, "n_groups": n0p, "n_src_rows": TAB0},
        {"f": 2, "n_groups": n1p, "n_src_rows": n0p},
        {"f": 4, "n_groups": n2p, "n_src_rows": n1p},
        {"f": 2, "n_groups": OUT_SIZES[3], "n_src_rows": n2p},
    ]
    S_ENC = N_VARS // P  # encode slots per partition

    nc = bacc.Bacc("TRN2", target_bir_lowering=False, debug=False)
    x = nc.dram_tensor("x", [P, S_ENC * B], f32, kind="ExternalInput")
    idx_in = [
        nc.dram_tensor(
            f"idx{l}", [P, s["f"] * s["n_groups"] // 16], i16, kind="ExternalInput"
        )
        for l, s in enumerate(specs)
    ]
    out = nc.dram_tensor("out", [OUT_SIZES[3], B], f32, kind="ExternalOutput")

    with tile.TileContext(nc) as tc:
        with (
            tc.tile_pool(name="dram", bufs=1, space="DRAM") as dpool,
            tc.tile_pool(name="sb", bufs=4) as gp,
            tc.tile_pool(name="hb", bufs=3) as hp,
            tc.tile_pool(name="tmp", bufs=2) as tp,
            tc.tile_pool(name="ix", bufs=1) as ixp,
        ):
            tables = [
                dpool.tile([s["n_src_rows"], B], f32, name=f"t{l}", tag=f"t{l}")
                for l, s in enumerate(specs)
            ]

            # --- index list loads ---
            ix_t = []
            for l, s in enumerate(specs):
                t = ixp.tile([P, s["f"] * s["n_groups"] // 16], i16, tag=f"ix{l}")
                nc.sync.dma_start(t[:], idx_in[l][:])
                ix_t.append(t)

            # Preload the combined Exp+Ln activation table once; the
            # insert_act_table_loads pass then finds every Exp/Ln already
            # covered and inserts no per-instruction reloads (1283ns each).
            ACT_SET_LN_EXP = 6  # natural_log_exp_and_others
            nc.scalar.add_instruction(
                mybir.InstLoadActFuncSet(
                    name=nc.get_next_instruction_name(),
                    ins=[],
                    outs=[],
                    act_func_set_id=ACT_SET_LN_EXP,
                )
            )

            # --- encode: pos rows at 2+2i, neg rows at 3+2i, zeros at rows
            # 0-1. Partition p computes vars p*S_ENC .. p*S_ENC+S_ENC-1 so
            # the interleaved pos/neg store is one contiguous run/partition.
            iv = gp.tile([P, S_ENC, 2, B], f32, tag="g")
            nc.sync.dma_start(
                iv[:][:, :, 0, :], x[:].rearrange("p (s b) -> p s b", b=B)
            )
            et = hp.tile([P, S_ENC, B], f32, tag="h")
            nc.scalar.activation(et[:], iv[:][:, :, 0, :], Act.Exp)
            nc.scalar.activation(iv[:][:, :, 1, :], et[:], Act.Ln, scale=-1.0, bias=1.0)
            nc.sync.dma_start(
                tables[0][:][2:, :].rearrange("(p s k) b -> p (s k b)", p=P, k=2),
                iv[:].rearrange("p s k b -> p (s k b)"),
            )
            # rows 0 (-inf in the reference, never gathered) and 1 (zeros)
            z = ixp.tile([2, B], f32, tag="z")
            nc.vector.memset(z[:], 0.0)
            nc.sync.dma_start(tables[0][:][0:2, :], z[:])

            # --- gather + segment-reduce layers ---
            for l, s in enumerate(specs):
                f, n_groups = s["f"], s["n_groups"]
                n_edges = n_groups * f
                C = n_groups // P
                src_ap = tables[l][:]
                dst_full = (
                    tables[l + 1][:] if l + 1 < len(specs) else out[:]
                ).rearrange("(p C) b -> p C b", p=P)
                c_off = 0
                e_off = 0
                for ch in _chunk_sizes(n_edges, f):
                    S = ch // P  # slots per chunk
                    Csub = S // f  # groups per partition per chunk
                    g = gp.tile([P, S, B], f32, tag="g")
                    nc.gpsimd.dma_gather(
                        g[:],
                        src_ap,
                        ix_t[l][:, e_off // 16 : (e_off + ch) // 16],
                        ch,
                        ch,
                        B,
                        single_packet=False,
                    )
                    v = g[:].rearrange("p (c k) b -> p c k b", k=f)
                    h = hp.tile([P, Csub, B], f32, tag="h")
                    if f == 4:
                        s01 = tp.tile([P, Csub, B], f32, tag="m")
                        s23 = tp.tile([P, Csub, B], f32, tag="n")
                        nc.vector.tensor_add(s01[:], v[:, :, 0, :], v[:, :, 1, :])
                        nc.vector.tensor_add(s23[:], v[:, :, 2, :], v[:, :, 3, :])
                        nc.vector.tensor_add(h[:], s01[:], s23[:])
                    elif l == 1:
                        # logsumexp(a,b) = a + ln(1+exp(b-a)); |b-a| < 40 so
                        # exp stays in f32 range.
                        d = tp.tile([P, Csub, B], f32, tag="m")
                        sp = tp.tile([P, Csub, B], f32, tag="n")
                        nc.vector.tensor_tensor(
                            d[:], v[:, :, 1, :], v[:, :, 0, :], op=Alu.subtract
                        )
                        nc.scalar.activation(d[:], d[:], Act.Exp)
                        nc.scalar.activation(sp[:], d[:], Act.Ln, bias=1.0)
                        nc.vector.tensor_add(h[:], v[:, :, 0, :], sp[:])
                    else:
                        # wider dynamic range: logsumexp = max + ln(1+exp(min-max))
                        m = tp.tile([P, Csub, B], f32, tag="m")
                        mn = tp.tile([P, Csub, B], f32, tag="n")
                        sp = tp.tile([P, Csub, B], f32, tag="sp")
                        nc.vector.tensor_tensor(
                            m[:], v[:, :, 0, :], v[:, :, 1, :], op=Alu.max
                        )
                        nc.vector.tensor_tensor(
                            mn[:], v[:, :, 0, :], v[:, :, 1, :], op=Alu.min
                        )
                        nc.vector.tensor_tensor(
                            mn[:], mn[:], m[:], op=Alu.subtract
                        )
                        nc.scalar.activation(mn[:], mn[:], Act.Exp)
                        nc.scalar.activation(sp[:], mn[:], Act.Ln, bias=1.0)
                        nc.vector.tensor_add(h[:], m[:], sp[:])
                    nc.sync.dma_start(dst_full[:, c_off : c_off + Csub, :], h[:])
                    c_off += Csub
                    e_off += ch
    nc.compile()
    return nc


def host_prep(x, ptrs_list, seg_list):
    """Host-side sharding + pruning + index preprocessing -> per-core maps."""
    x = np.asarray(x, dtype=np.float32)
    for l, (n_out, f) in enumerate(zip(OUT_SIZES, FANINS)):
        seg = np.asarray(seg_list[l]).astype(np.int64)
        expected = np.repeat(np.arange(n_out, dtype=np.int64), f)
        assert np.array_equal(seg, expected), f"layer {l}: non-uniform segments"

    layers = plan(ptrs_list)
    idx_maps = {
        f"idx{l}": reorder_wrap(s["src"], s["f"], s["n_groups"])
        for l, s in enumerate(layers)
    }

    batch = x.shape[1]
    bpc = batch // NCORES
    in_maps = []
    for i in range(NCORES):
        xs = x[:, i * bpc : (i + 1) * bpc]
        # partition p holds vars p*S_ENC .. p*S_ENC+S_ENC-1 (natural order)
        xv = np.ascontiguousarray(xs).reshape(P, -1)
        in_maps.append({"x": xv, **idx_maps})
    return in_maps


_CACHE = {}


def _get_nc(counts=None):
    if counts is None:
        counts = _CACHE.get("counts")
        assert counts is not None, "call kernel() first"
    if _CACHE.get("counts") != counts:
        _CACHE["nc"] = build_nc(counts)
        _CACHE["counts"] = counts
    return _CACHE["nc"]


def kernel(x, ptrs0, seg0, ptrs1, seg1, ptrs2, seg2, ptrs3, seg3):
    from concourse.bass_utils import run_bass_kernel_spmd

    ptrs_list = [ptrs0, ptrs1, ptrs2, ptrs3]
    layers = plan(ptrs_list)
    counts = tuple(s["n_groups"] for s in layers[:3])
    nc = _get_nc(counts)
    in_maps = host_prep(x, ptrs_list, [seg0, seg1, seg2, seg3])
    res = run_bass_kernel_spmd(nc, in_maps, core_ids=list(range(NCORES)))
    outs = [r["out"] for r in res.results]
    return np.concatenate(outs, axis=1)


# revision 6
# speedup vs baseline: 1.5155x; 1.0554x over previous
"""Trainium2 Bass kernel for the sum-product "knowledge layer" network.

Computation (see problem reference):
  h0 = encode(x): 8194-row table [-inf, 0, pos0, neg0, pos1, neg1, ...]
       with pos = x (log-probs), neg = log(1 - exp(x)), per batch column.
  4 alternating layers, each: gather rows by ptrs, then segment-reduce over
  contiguous fanin groups (fanin 4 sum-of-logs "product" layers, fanin 2
  logsumexp "sum" layers).

Strategy (pure batch data-parallelism, 8 NeuronCores):
  - Shard the 512 batch columns 8 ways -> 64 columns per core.
  - Per core every tensor lives in DRAM as [rows, 64] fp32; one row = 256B.
  - Gathers use the SWDGE dma_gather instruction: int16 index list in SBUF,
    each index pulls one 256B row from the DRAM table; index list position j
    lands at SBUF partition j%128, free slot j//128.
  - DAG pruning (host side, per input set): working back from the 2048
    output rows, only rows actually referenced downstream are computed.
    Each layer's output table is compacted (relabeled); the relabeling is
    folded into the next layer's gather indices. This removes ~40% of all
    gather descriptors (the dominant cost) and shrinks the table stores.
  - Cross-layer software pipelining: each layer's output rows are produced
    in chunk order (chunk ci of layer l stores table rows
    [ci*G, ci*G + P*Csub), row = base + p*Csub + cc), consumer groups are
    sorted by the maximum source row they reference, and every gather
    chunk's source AP is narrowed to the exact table prefix it needs.  The
    tile framework then only serializes a gather against the stores that
    overlap its prefix, so layer l+1's early chunks run while layer l's
    tail is still in flight.  The encode is likewise chunked (vars are
    laid out slot-major: var v lives at partition v%128, slot v//128, so
    encode chunk j fills table0 rows [2+256*j*SE, ...)).
  - Sum layers: logsumexp(a,b) via max + ln(1+exp(min-max)) on DVE + ACT,
    with the Exp+Ln activation table preloaded once (set 6) so the
    compiler inserts no per-instruction table reloads.
"""

import numpy as np

P = 128
B = 64  # batch columns per core
NCORES = 8
N_VARS = 4096
BATCH = 512
TAB0 = 2 * N_VARS + 2  # 8194
OUT_SIZES = [16384, 8192, 4096, 2048]
FANINS = [4, 2, 4, 2]
CHUNK = 8192  # max gather indices per dma_gather instruction
S_ENC = N_VARS // P  # 32 encode slots per partition
ENC_CHUNKS = 4
SE = S_ENC // ENC_CHUNKS  # slots per encode chunk


def _pad_groups(n):
    return -(-n // P) * P


def _chunk_group_counts(n_groups, f):
    """Groups per dma_gather chunk: full CHUNK-edge chunks + remainder."""
    per = CHUNK // f
    out = []
    left = n_groups
    while left > 0:
        c = min(per, left)
        assert (c * f) % (f * P) == 0
        out.append(c)
        left -= c
    return out


def plan(ptrs_list):
    """Prune the DAG bottom-up, compact + readiness-order each layer.

    Returns a list of per-layer dicts:
      f: fanin
      n_groups: padded compacted group count (= rows of the next table)
      n_src_rows: row count of the table this layer gathers from
      chunks: list of (n_groups_in_chunk, src_prefix_rows)
      edge_src: per-edge source-row ids, grouped by *production* order
                (length f*n_groups)
    Layer l's output row r is produced by chunk ci at position
    r = base_ci + p*Csub + cc; groups are ordered by the max source row
    they reference so early chunks only need early source prefixes.
    """
    p0, p1, p2, p3 = [np.asarray(p).astype(np.int64) for p in ptrs_list]
    e3 = p3.reshape(-1, 2)
    used2 = np.unique(e3)
    e2 = p2.reshape(-1, 4)[used2]
    used1 = np.unique(e2)
    e1 = p1.reshape(-1, 2)[used1]
    used0 = np.unique(e1)
    e0 = p0.reshape(-1, 4)[used0]

    def relabel(edges, used):
        return np.searchsorted(used, edges)

    raw = [
        (e0, used0.size),
        (relabel(e1, used0), used1.size),
        (relabel(e2, used1), used2.size),
        (relabel(e3, used2), OUT_SIZES[3]),
    ]

    layers = []
    prev_rows = TAB0
    # prod_map[g_compact] = production row id of layer-l group g (built as
    # each layer is ordered); identity for the t0 "layer" handled inline.
    prod_map = None
    for l, (edges, n_exact) in enumerate(raw):
        f = edges.shape[1]
        n_pad = _pad_groups(n_exact)
        src = np.zeros((n_pad, f), dtype=np.int64)
        src[:n_exact] = edges if prod_map is None else prod_map[edges]
        if l == 0:
            # readiness of a t0 row: encode order. rows 0/1 ready first,
            # row 2+2v+k ready with var slot v//128.
            ready = np.maximum(src - 2, 0) // 2 // P
        else:
            ready = src  # source production row id
        if l < 3:
            order = np.argsort(ready.max(axis=1), kind="stable")
        else:
            order = np.arange(n_pad)  # output rows keep natural order
        src = src[order]
        # production row of original compacted group id
        new_prod = np.empty(n_pad, dtype=np.int64)
        new_prod[order] = np.arange(n_pad)
        prod_map = new_prod

        chunks = []
        g_off = 0
        for gc in _chunk_group_counts(n_pad, f):
            m = int(src[g_off : g_off + gc].max()) + 1
            chunks.append((gc, m))
            g_off += gc
        layers.append(
            {
                "f": f,
                "n_groups": n_pad,
                "n_src_rows": prev_rows,
                "chunks": chunks,
                "edge_src": src.ravel(),
            }
        )
        prev_rows = n_pad
    return layers


def reorder_wrap(layer):
    """Permute per-edge source ids into dma_gather order and wrap into the
    int16 [128, n_edges//16] SBUF layout (position j -> [j%16, j//16],
    replicated across the 8 gpsimd cores' 16-partition groups).

    Edge position j of chunk ci maps to partition p=j%128, slot=j//128,
    cc=slot//f, k=slot%f, production row = base_ci + p*Csub + cc (layers
    0-2) or p*C + cc (layer 3, single chunk, natural order)."""
    f = layer["f"]
    src = layer["edge_src"]
    out = np.empty(layer["n_groups"] * f, dtype=np.int64)
    base = 0
    e_off = 0
    for gc, _m in layer["chunks"]:
        csub = gc // P
        n_e = gc * f
        j = np.arange(n_e)
        p = j % P
        slot = j // P
        cc = slot // f
        k = slot % f
        row = base + p * csub + cc
        out[e_off : e_off + n_e] = src[row * f + k]
        base += gc
        e_off += n_e
    assert out.max() < 2**15 and out.min() >= 0
    out = out.astype(np.int16)
    return np.ascontiguousarray(np.tile(out.reshape(-1, 16).T, (8, 1)))


def build_nc(meta):
    """meta: tuple of per-layer (f, n_groups, n_src_rows, chunks-tuple)."""
    import concourse.bacc as bacc
    import concourse.mybir as mybir
    import concourse.tile as tile

    f32 = mybir.dt.float32
    i16 = mybir.dt.int16
    Alu = mybir.AluOpType
    Act = mybir.ActivationFunctionType

    specs = [
        {"f": f, "n_groups": n, "n_src_rows": s, "chunks": ch}
        for (f, n, s, ch) in meta
    ]

    nc = bacc.Bacc("TRN2", target_bir_lowering=False, debug=False)
    x = nc.dram_tensor("x", [P, S_ENC * B], f32, kind="ExternalInput")
    idx_in = [
        nc.dram_tensor(
            f"idx{l}", [P, s["f"] * s["n_groups"] // 16], i16, kind="ExternalInput"
        )
        for l, s in enumerate(specs)
    ]
    out = nc.dram_tensor("out", [OUT_SIZES[3], B], f32, kind="ExternalOutput")

    with tile.TileContext(nc) as tc:
        with (
            tc.tile_pool(name="dram", bufs=1, space="DRAM") as dpool,
            tc.tile_pool(name="sb", bufs=4) as gp,
            tc.tile_pool(name="enc", bufs=ENC_CHUNKS) as ep,
            tc.tile_pool(name="hb", bufs=3) as hp,
            tc.tile_pool(name="tmp", bufs=2) as tp,
            tc.tile_pool(name="ix", bufs=1) as ixp,
        ):
            tables = [
                dpool.tile([s["n_src_rows"], B], f32, name=f"t{l}", tag=f"t{l}")
                for l, s in enumerate(specs)
            ]

            # Preload the combined Exp+Ln activation table once; the
            # insert_act_table_loads pass then finds every Exp/Ln already
            # covered and inserts no per-instruction reloads (1283ns each).
            ACT_SET_LN_EXP = 6  # natural_log_exp_and_others
            nc.scalar.add_instruction(
                mybir.InstLoadActFuncSet(
                    name=nc.get_next_instruction_name(),
                    ins=[],
                    outs=[],
                    act_func_set_id=ACT_SET_LN_EXP,
                )
            )

            # table0 rows 0 (-inf in the reference, never gathered) and 1
            # (zeros). Store first so the row prefix [0,2) is ready.
            z = ixp.tile([2, B], f32, tag="z")
            nc.vector.memset(z[:], 0.0)
            nc.sync.dma_start(tables[0][:][0:2, :], z[:])

            # --- encode, chunked: var v sits at partition v%128, slot
            # v//128; pos row 2+2v, neg row 3+2v.  Chunk j covers slots
            # [j*SE,(j+1)*SE) = rows [2+256*j*SE, 2+256*(j+1)*SE), a row
            # prefix, so layer-0 gather chunks can start before the whole
            # encode finishes.
            xv = x[:].rearrange("p (s b) -> p s b", b=B)
            for j in range(ENC_CHUNKS):
                iv = ep.tile([P, SE, 2, B], f32, tag="enc")
                nc.sync.dma_start(iv[:][:, :, 0, :], xv[:, j * SE : (j + 1) * SE, :])
                et = hp.tile([P, SE, B], f32, tag="h")
                nc.scalar.activation(et[:], iv[:][:, :, 0, :], Act.Exp)
                nc.scalar.activation(
                    iv[:][:, :, 1, :], et[:], Act.Ln, scale=-1.0, bias=1.0
                )
                r0 = 2 + 2 * P * SE * j
                r1 = 2 + 2 * P * SE * (j + 1)
                # row = r0 + 256*s + 2*p + k  ->  "(s p k) b -> p (s k b)"
                nc.sync.dma_start(
                    tables[0][:][r0:r1, :].rearrange("(s p k) b -> p s k b", p=P, k=2),
                    iv[:],
                )

            # --- index list loads ---
            ix_t = []
            for l, s in enumerate(specs):
                t = ixp.tile([P, s["f"] * s["n_groups"] // 16], i16, tag=f"ix{l}")
                nc.sync.dma_start(t[:], idx_in[l][:])
                ix_t.append(t)

            # --- gather + segment-reduce layers ---
            for l, s in enumerate(specs):
                f = s["f"]
                dst_tile = tables[l + 1][:] if l + 1 < len(specs) else out[:]
                g_off = 0
                e_off = 0
                for gc, m_src in s["chunks"]:
                    csub = gc // P
                    ch = gc * f
                    S = ch // P
                    g = gp.tile([P, S, B], f32, tag="g")
                    nc.gpsimd.dma_gather(
                        g[:],
                        tables[l][:][0:m_src, :],
                        ix_t[l][:, e_off // 16 : (e_off + ch) // 16],
                        ch,
                        ch,
                        B,
                        single_packet=False,
                    )
                    v = g[:].rearrange("p (c k) b -> p c k b", k=f)
                    h = hp.tile([P, csub, B], f32, tag="h")
                    if f == 4:
                        s01 = tp.tile([P, csub, B], f32, tag="m")
                        s23 = tp.tile([P, csub, B], f32, tag="n")
                        nc.vector.tensor_add(s01[:], v[:, :, 0, :], v[:, :, 1, :])
                        nc.vector.tensor_add(s23[:], v[:, :, 2, :], v[:, :, 3, :])
                        nc.vector.tensor_add(h[:], s01[:], s23[:])
                    elif l == 1:
                        # logsumexp(a,b) = a + ln(1+exp(b-a)); |b-a| < 40
                        # here so exp stays in f32 range.
                        d = tp.tile([P, csub, B], f32, tag="m")
                        sp = tp.tile([P, csub, B], f32, tag="n")
                        nc.vector.tensor_tensor(
                            d[:], v[:, :, 1, :], v[:, :, 0, :], op=Alu.subtract
                        )
                        nc.scalar.activation(d[:], d[:], Act.Exp)
                        nc.scalar.activation(sp[:], d[:], Act.Ln, bias=1.0)
                        nc.vector.tensor_add(h[:], v[:, :, 0, :], sp[:])
                    else:
                        # wider range: logsumexp = max + ln(1+exp(min-max))
                        m = tp.tile([P, csub, B], f32, tag="m")
                        mn = tp.tile([P, csub, B], f32, tag="n")
                        sp = tp.tile([P, csub, B], f32, tag="sp")
                        nc.vector.tensor_tensor(
                            m[:], v[:, :, 0, :], v[:, :, 1, :], op=Alu.max
                        )
                        nc.vector.tensor_tensor(
                            mn[:], v[:, :, 0, :], v[:, :, 1, :], op=Alu.min
                        )
                        nc.vector.tensor_tensor(mn[:], mn[:], m[:], op=Alu.subtract)
                        nc.scalar.activation(mn[:], mn[:], Act.Exp)
                        nc.scalar.activation(sp[:], mn[:], Act.Ln, bias=1.0)
                        nc.vector.tensor_add(h[:], m[:], sp[:])
                    # chunk ci produces rows [g_off, g_off + P*csub):
                    # row = g_off + p*csub + cc
                    nc.sync.dma_start(
                        dst_tile[g_off : g_off + P * csub, :].rearrange(
                            "(p c) b -> p (c b)", p=P
                        ),
                        h[:].rearrange("p c b -> p (c b)"),
                    )
                    g_off += P * csub
                    e_off += ch
    nc.compile()
    return nc


def host_prep(x, ptrs_list, seg_list):
    """Host-side sharding + pruning + index preprocessing -> per-core maps."""
    x = np.asarray(x, dtype=np.float32)
    for l, (n_out, f) in enumerate(zip(OUT_SIZES, FANINS)):
        seg = np.asarray(seg_list[l]).astype(np.int64)
        expected = np.repeat(np.arange(n_out, dtype=np.int64), f)
        assert np.array_equal(seg, expected), f"layer {l}: non-uniform segments"

    layers = plan(ptrs_list)
    idx_maps = {f"idx{l}": reorder_wrap(s) for l, s in enumerate(layers)}

    batch = x.shape[1]
    bpc = batch // NCORES
    in_maps = []
    for i in range(NCORES):
        xs = x[:, i * bpc : (i + 1) * bpc]
        # partition p, slot s holds var s*128+p (slot-major var layout)
        xv = np.ascontiguousarray(
            xs.reshape(S_ENC, P, bpc).transpose(1, 0, 2)
        ).reshape(P, -1)
        in_maps.append({"x": xv, **idx_maps})
    return in_maps


def _meta(layers):
    return tuple(
        (s["f"], s["n_groups"], s["n_src_rows"], tuple(s["chunks"])) for s in layers
    )


_CACHE = {}


def _get_nc(meta=None):
    if meta is None:
        meta = _CACHE.get("meta")
        assert meta is not None, "call kernel() first"
    if _CACHE.get("meta") != meta:
        _CACHE["nc"] = build_nc(meta)
        _CACHE["meta"] = meta
    return _CACHE["nc"]


def kernel(x, ptrs0, seg0, ptrs1, seg1, ptrs2, seg2, ptrs3, seg3):
    from concourse.bass_utils import run_bass_kernel_spmd

    ptrs_list = [ptrs0, ptrs1, ptrs2, ptrs3]
    layers = plan(ptrs_list)
    nc = _get_nc(_meta(layers))
    in_maps = host_prep(x, ptrs_list, [seg0, seg1, seg2, seg3])
    res = run_bass_kernel_spmd(nc, in_maps, core_ids=list(range(NCORES)))
    outs = [r["out"] for r in res.results]
    return np.concatenate(outs, axis=1)


# revision 7
# speedup vs baseline: 1.7405x; 1.1484x over previous
"""Trainium2 Bass kernel for the sum-product "knowledge layer" network.

Computation (see problem reference):
  h0 = encode(x): 8194-row table [-inf, 0, pos0, neg0, pos1, neg1, ...]
       with pos = x (log-probs), neg = log(1 - exp(x)), per batch column.
  4 alternating layers, each: gather rows by ptrs, then segment-reduce over
  contiguous fanin groups (fanin 4 sum-of-logs "product" layers, fanin 2
  logsumexp "sum" layers).

Strategy (pure batch data-parallelism, 8 NeuronCores):
  - Shard the 512 batch columns 8 ways -> 64 columns per core.
  - Per core every tensor lives in DRAM as [rows, 64] fp32; one row = 256B.
  - Gathers use the SWDGE dma_gather instruction: int16 index list in SBUF,
    each index pulls one 256B row from the DRAM table; index list position j
    lands at SBUF partition j%128, free slot j//128.
  - DAG pruning (host side, per input set): working back from the 2048
    output rows, only rows actually referenced downstream are computed.
    Each layer's output table is compacted (relabeled); the relabeling is
    folded into the next layer's gather indices. This removes ~40% of all
    gather descriptors (the dominant cost) and shrinks the table stores.
  - Cross-layer software pipelining: each layer's output rows are produced
    in chunk order (chunk ci of layer l stores table rows
    [ci*G, ci*G + P*Csub), row = base + p*Csub + cc), consumer groups are
    sorted by the maximum source row they reference, and every gather
    chunk's source AP is narrowed to the exact table prefix it needs.  The
    tile framework then only serializes a gather against the stores that
    overlap its prefix, so layer l+1's early chunks run while layer l's
    tail is still in flight.  The encode is likewise chunked (vars are
    laid out slot-major: var v lives at partition v%128, slot v//128, so
    encode chunk j fills table0 rows [2+256*j*SE, ...)).
  - Sum layers: logsumexp(a,b) via max + ln(1+exp(min-max)) on DVE + ACT,
    with the Exp+Ln activation table preloaded once (set 6) so the
    compiler inserts no per-instruction table reloads.
"""

import numpy as np

P = 128
B = 64  # batch columns per core
NCORES = 8
N_VARS = 4096
BATCH = 512
TAB0 = 2 * N_VARS + 2  # 8194
OUT_SIZES = [16384, 8192, 4096, 2048]
FANINS = [4, 2, 4, 2]
CHUNK = 8192  # max gather indices per dma_gather instruction
S_ENC = N_VARS // P  # 32 encode slots per partition
ENC_CHUNKS = 4
SE = S_ENC // ENC_CHUNKS  # slots per encode chunk


def _pad_groups(n):
    return -(-n // P) * P


def _chunk_group_counts(n_groups, f):
    """Groups per dma_gather chunk.  Chunk sizes ramp up at the start of a
    layer (small first chunk -> its SWDGE desc-gen finishes quickly once the
    source prefix lands) and ramp down at the end (short compute tail, so
    the last table store lands early and the next layer unblocks sooner).
    Every chunk is a multiple of 128 groups (= f*128 edges)."""
    g = P  # group granule
    rem = n_groups
    tail = []
    for s in (512, 512, 1024, 2048):
        s = max(s // f, g)
        if rem >= s + g:
            tail.append(s)
            rem -= s
    head = []
    for s in (512, 1024, 2048, 4096):
        s = max(s // f, g)
        if rem >= s + g:
            head.append(s)
            rem -= s
    mid = []
    per = CHUNK // f
    while rem > 0:
        s = min(per, rem)
        mid.append(s)
        rem -= s
    return head + mid + tail[::-1]


def plan(ptrs_list):
    """Prune the DAG bottom-up, compact + readiness-order each layer.

    Returns a list of per-layer dicts:
      f: fanin
      n_groups: padded compacted group count (= rows of the next table)
      n_src_rows: row count of the table this layer gathers from
      chunks: list of (n_groups_in_chunk, src_prefix_rows)
      edge_src: per-edge source-row ids, grouped by *production* order
                (length f*n_groups)
    Layer l's output row r is produced by chunk ci at position
    r = base_ci + p*Csub + cc; groups are ordered by the max source row
    they reference so early chunks only need early source prefixes.
    """
    p0, p1, p2, p3 = [np.asarray(p).astype(np.int64) for p in ptrs_list]
    e3 = p3.reshape(-1, 2)
    used2 = np.unique(e3)
    e2 = p2.reshape(-1, 4)[used2]
    used1 = np.unique(e2)
    e1 = p1.reshape(-1, 2)[used1]
    used0 = np.unique(e1)
    e0 = p0.reshape(-1, 4)[used0]

    def relabel(edges, used):
        return np.searchsorted(used, edges)

    raw = [
        (e0, used0.size),
        (relabel(e1, used0), used1.size),
        (relabel(e2, used1), used2.size),
        (relabel(e3, used2), OUT_SIZES[3]),
    ]

    layers = []
    prev_rows = TAB0
    # prod_map[g_compact] = production row id of layer-l group g (built as
    # each layer is ordered); identity for the t0 "layer" handled inline.
    prod_map = None
    for l, (edges, n_exact) in enumerate(raw):
        f = edges.shape[1]
        n_pad = _pad_groups(n_exact)
        src = np.zeros((n_pad, f), dtype=np.int64)
        src[:n_exact] = edges if prod_map is None else prod_map[edges]
        if l == 0:
            # readiness of a t0 row: encode order. rows 0/1 ready first,
            # row 2+2v+k ready with var slot v//128.
            ready = np.maximum(src - 2, 0) // 2 // P
        else:
            ready = src  # source production row id
        if l < 3:
            order = np.argsort(ready.max(axis=1), kind="stable")
        else:
            order = np.arange(n_pad)  # output rows keep natural order
        src = src[order]
        # production row of original compacted group id
        new_prod = np.empty(n_pad, dtype=np.int64)
        new_prod[order] = np.arange(n_pad)
        prod_map = new_prod

        chunks = []
        g_off = 0
        for gc in _chunk_group_counts(n_pad, f):
            m = int(src[g_off : g_off + gc].max()) + 1
            chunks.append((gc, m))
            g_off += gc
        layers.append(
            {
                "f": f,
                "n_groups": n_pad,
                "n_src_rows": prev_rows,
                "chunks": chunks,
                "edge_src": src.ravel(),
            }
        )
        prev_rows = n_pad
    return layers


def reorder_wrap(layer):
    """Permute per-edge source ids into dma_gather order and wrap into the
    int16 [128, n_edges//16] SBUF layout (position j -> [j%16, j//16],
    replicated across the 8 gpsimd cores' 16-partition groups).

    Edge position j of chunk ci maps to partition p=j%128, slot=j//128,
    cc=slot//f, k=slot%f, production row = base_ci + p*Csub + cc (layers
    0-2) or p*C + cc (layer 3, single chunk, natural order)."""
    f = layer["f"]
    src = layer["edge_src"]
    out = np.empty(layer["n_groups"] * f, dtype=np.int64)
    base = 0
    e_off = 0
    for gc, _m in layer["chunks"]:
        csub = gc // P
        n_e = gc * f
        j = np.arange(n_e)
        p = j % P
        slot = j // P
        cc = slot // f
        k = slot % f
        row = base + p * csub + cc
        out[e_off : e_off + n_e] = src[row * f + k]
        base += gc
        e_off += n_e
    assert out.max() < 2**15 and out.min() >= 0
    out = out.astype(np.int16)
    return np.ascontiguousarray(np.tile(out.reshape(-1, 16).T, (8, 1)))


def build_nc(meta):
    """meta: tuple of per-layer (f, n_groups, n_src_rows, chunks-tuple)."""
    import concourse.bacc as bacc
    import concourse.mybir as mybir
    import concourse.tile as tile

    f32 = mybir.dt.float32
    i16 = mybir.dt.int16
    Alu = mybir.AluOpType
    Act = mybir.ActivationFunctionType

    specs = [
        {"f": f, "n_groups": n, "n_src_rows": s, "chunks": ch}
        for (f, n, s, ch) in meta
    ]

    nc = bacc.Bacc("TRN2", target_bir_lowering=False, debug=False)
    x = nc.dram_tensor("x", [P, S_ENC * B], f32, kind="ExternalInput")
    idx_in = [
        nc.dram_tensor(
            f"idx{l}", [P, s["f"] * s["n_groups"] // 16], i16, kind="ExternalInput"
        )
        for l, s in enumerate(specs)
    ]
    out = nc.dram_tensor("out", [OUT_SIZES[3], B], f32, kind="ExternalOutput")

    with tile.TileContext(nc) as tc:
        with (
            tc.tile_pool(name="dram", bufs=1, space="DRAM") as dpool,
            tc.tile_pool(name="sb", bufs=4) as gp,
            tc.tile_pool(name="enc", bufs=ENC_CHUNKS) as ep,
            tc.tile_pool(name="hb", bufs=3) as hp,
            tc.tile_pool(name="tmp", bufs=2) as tp,
            tc.tile_pool(name="ix", bufs=1) as ixp,
        ):
            tables = [
                dpool.tile([s["n_src_rows"], B], f32, name=f"t{l}", tag=f"t{l}")
                for l, s in enumerate(specs)
            ]

            # Preload the combined Exp+Ln activation table once; the
            # insert_act_table_loads pass then finds every Exp/Ln already
            # covered and inserts no per-instruction reloads (1283ns each).
            ACT_SET_LN_EXP = 6  # natural_log_exp_and_others
            nc.scalar.add_instruction(
                mybir.InstLoadActFuncSet(
                    name=nc.get_next_instruction_name(),
                    ins=[],
                    outs=[],
                    act_func_set_id=ACT_SET_LN_EXP,
                )
            )

            # table0 rows 0 (-inf in the reference, never gathered) and 1
            # (zeros). Store first so the row prefix [0,2) is ready.
            z = ixp.tile([2, B], f32, tag="z")
            nc.vector.memset(z[:], 0.0)
            nc.sync.dma_start(tables[0][:][0:2, :], z[:])

            # --- encode, chunked: var v sits at partition v%128, slot
            # v//128; pos row 2+2v, neg row 3+2v.  Chunk j covers slots
            # [j*SE,(j+1)*SE) = rows [2+256*j*SE, 2+256*(j+1)*SE), a row
            # prefix, so layer-0 gather chunks can start before the whole
            # encode finishes.
            xv = x[:].rearrange("p (s b) -> p s b", b=B)
            for j in range(ENC_CHUNKS):
                iv = ep.tile([P, SE, 2, B], f32, tag="enc")
                nc.sync.dma_start(iv[:][:, :, 0, :], xv[:, j * SE : (j + 1) * SE, :])
                et = hp.tile([P, SE, B], f32, tag="h")
                nc.scalar.activation(et[:], iv[:][:, :, 0, :], Act.Exp)
                nc.scalar.activation(
                    iv[:][:, :, 1, :], et[:], Act.Ln, scale=-1.0, bias=1.0
                )
                r0 = 2 + 2 * P * SE * j
                r1 = 2 + 2 * P * SE * (j + 1)
                # row = r0 + 256*s + 2*p + k  ->  "(s p k) b -> p (s k b)"
                nc.sync.dma_start(
                    tables[0][:][r0:r1, :].rearrange("(s p k) b -> p s k b", p=P, k=2),
                    iv[:],
                )

            # --- index list loads ---
            ix_t = []
            for l, s in enumerate(specs):
                t = ixp.tile([P, s["f"] * s["n_groups"] // 16], i16, tag=f"ix{l}")
                nc.sync.dma_start(t[:], idx_in[l][:])
                ix_t.append(t)

            # --- gather + segment-reduce layers ---
            for l, s in enumerate(specs):
                f = s["f"]
                dst_tile = tables[l + 1][:] if l + 1 < len(specs) else out[:]
                g_off = 0
                e_off = 0
                for gc, m_src in s["chunks"]:
                    csub = gc // P
                    ch = gc * f
                    S = ch // P
                    g = gp.tile([P, S, B], f32, tag="g")
                    nc.gpsimd.dma_gather(
                        g[:],
                        tables[l][:][0:m_src, :],
                        ix_t[l][:, e_off // 16 : (e_off + ch) // 16],
                        ch,
                        ch,
                        B,
                        single_packet=False,
                    )
                    v = g[:].rearrange("p (c k) b -> p c k b", k=f)
                    h = hp.tile([P, csub, B], f32, tag="h")
                    if f == 4:
                        s01 = tp.tile([P, csub, B], f32, tag="m")
                        s23 = tp.tile([P, csub, B], f32, tag="n")
                        nc.vector.tensor_add(s01[:], v[:, :, 0, :], v[:, :, 1, :])
                        nc.vector.tensor_add(s23[:], v[:, :, 2, :], v[:, :, 3, :])
                        nc.vector.tensor_add(h[:], s01[:], s23[:])
                    elif l == 1:
                        # logsumexp(a,b) = a + ln(1+exp(b-a)); |b-a| < 40
                        # here so exp stays in f32 range.
                        d = tp.tile([P, csub, B], f32, tag="m")
                        sp = tp.tile([P, csub, B], f32, tag="n")
                        nc.vector.tensor_tensor(
                            d[:], v[:, :, 1, :], v[:, :, 0, :], op=Alu.subtract
                        )
                        nc.scalar.activation(d[:], d[:], Act.Exp)
                        nc.scalar.activation(sp[:], d[:], Act.Ln, bias=1.0)
                        nc.vector.tensor_add(h[:], v[:, :, 0, :], sp[:])
                    else:
                        # wider range: logsumexp = max + ln(1+exp(min-max))
                        m = tp.tile([P, csub, B], f32, tag="m")
                        mn = tp.tile([P, csub, B], f32, tag="n")
                        sp = tp.tile([P, csub, B], f32, tag="sp")
                        nc.vector.tensor_tensor(
                            m[:], v[:, :, 0, :], v[:, :, 1, :], op=Alu.max
                        )
                        nc.vector.tensor_tensor(
                            mn[:], v[:, :, 0, :], v[:, :, 1, :], op=Alu.min
                        )
                        nc.vector.tensor_tensor(mn[:], mn[:], m[:], op=Alu.subtract)
                        nc.scalar.activation(mn[:], mn[:], Act.Exp)
                        nc.scalar.activation(sp[:], mn[:], Act.Ln, bias=1.0)
                        nc.vector.tensor_add(h[:], m[:], sp[:])
                    # chunk ci produces rows [g_off, g_off + P*csub):
                    # row = g_off + p*csub + cc
                    nc.sync.dma_start(
                        dst_tile[g_off : g_off + P * csub, :].rearrange(
                            "(p c) b -> p (c b)", p=P
                        ),
                        h[:].rearrange("p c b -> p (c b)"),
                    )
                    g_off += P * csub
                    e_off += ch
    nc.compile()
    return nc


def host_prep(x, ptrs_list, seg_list):
    """Host-side sharding + pruning + index preprocessing -> per-core maps."""
    x = np.asarray(x, dtype=np.float32)
    for l, (n_out, f) in enumerate(zip(OUT_SIZES, FANINS)):
        seg = np.asarray(seg_list[l]).astype(np.int64)
        expected = np.repeat(np.arange(n_out, dtype=np.int64), f)
        assert np.array_equal(seg, expected), f"layer {l}: non-uniform segments"

    layers = plan(ptrs_list)
    idx_maps = {f"idx{l}": reorder_wrap(s) for l, s in enumerate(layers)}

    batch = x.shape[1]
    bpc = batch // NCORES
    in_maps = []
    for i in range(NCORES):
        xs = x[:, i * bpc : (i + 1) * bpc]
        # partition p, slot s holds var s*128+p (slot-major var layout)
        xv = np.ascontiguousarray(
            xs.reshape(S_ENC, P, bpc).transpose(1, 0, 2)
        ).reshape(P, -1)
        in_maps.append({"x": xv, **idx_maps})
    return in_maps


def _meta(layers):
    return tuple(
        (s["f"], s["n_groups"], s["n_src_rows"], tuple(s["chunks"])) for s in layers
    )


_CACHE = {}


def _get_nc(meta=None):
    if meta is None:
        meta = _CACHE.get("meta")
        assert meta is not None, "call kernel() first"
    if _CACHE.get("meta") != meta:
        _CACHE["nc"] = build_nc(meta)
        _CACHE["meta"] = meta
    return _CACHE["nc"]


def kernel(x, ptrs0, seg0, ptrs1, seg1, ptrs2, seg2, ptrs3, seg3):
    from concourse.bass_utils import run_bass_kernel_spmd

    ptrs_list = [ptrs0, ptrs1, ptrs2, ptrs3]
    layers = plan(ptrs_list)
    nc = _get_nc(_meta(layers))
    in_maps = host_prep(x, ptrs_list, [seg0, seg1, seg2, seg3])
    res = run_bass_kernel_spmd(nc, in_maps, core_ids=list(range(NCORES)))
    outs = [r["out"] for r in res.results]
    return np.concatenate(outs, axis=1)


# revision 8
# speedup vs baseline: 1.8706x; 1.0748x over previous
"""Trainium2 Bass kernel for the sum-product "knowledge layer" network.

Computation (see problem reference):
  h0 = encode(x): 8194-row table [-inf, 0, pos0, neg0, pos1, neg1, ...]
       with pos = x (log-probs), neg = log(1 - exp(x)), per batch column.
  4 alternating layers, each: gather rows by ptrs, then segment-reduce over
  contiguous fanin groups (fanin 4 sum-of-logs "product" layers, fanin 2
  logsumexp "sum" layers).

Strategy (pure batch data-parallelism, 8 NeuronCores):
  - Shard the 512 batch columns 8 ways -> 64 columns per core.
  - Per core every tensor lives in DRAM as [rows, 64] fp32; one row = 256B.
  - Gathers use the SWDGE dma_gather instruction: int16 index list in SBUF,
    each index pulls one 256B row from the DRAM table; index list position j
    lands at SBUF partition j%128, free slot j//128.
  - DAG pruning (host side, per input set): working back from the 2048
    output rows, only rows actually referenced downstream are computed.
  - Layer fusion: layers 0+1 fuse into stage A, layers 2+3 into stage B.
    A stage group gathers its 8 leaf rows (2 product groups x fanin 4),
    sums each quad on DVE, then logsumexps the pair - so the intermediate
    product table never exists in DRAM, removing its store and a pipeline
    boundary, at the cost of recomputing product rows referenced by more
    than one sum edge (~4% more gather descriptors).
  - Cross-layer software pipelining: stage A's output rows are produced in
    chunk order (chunk ci stores rows [base, base + P*csub), row = base +
    p*csub + cc), A groups are sorted by the max table-0 row they
    reference, and every gather chunk's source AP is narrowed to the exact
    table prefix it needs, so the tile framework only serializes a gather
    against the stores that overlap its prefix. The encode is likewise
    chunked (vars are laid out slot-major: var v lives at partition v%128,
    slot v//128, so encode chunk j fills a table-0 row prefix). Chunk
    sizes ramp up then down so desc-gen and compute tails stay short.
  - Sum reduction: logsumexp(a,b) = a + ln(1+exp(b-a)) in stage A (|b-a|
    < 40, f32-exp-safe) and max + ln(1+exp(min-max)) in stage B (wider
    range), with the Exp+Ln activation table preloaded once (set 6) so
    the compiler inserts no per-instruction table reloads.
"""

import numpy as np

P = 128
B = 64  # batch columns per core
NCORES = 8
N_VARS = 4096
BATCH = 512
TAB0 = 2 * N_VARS + 2  # 8194
OUT_SIZES = [16384, 8192, 4096, 2048]
FANINS = [4, 2, 4, 2]
FE = 8  # edges per fused group: 2 (sum fanin) x 4 (product fanin)
CHUNK = 8192  # max gather indices per dma_gather instruction
S_ENC = N_VARS // P  # 32 encode slots per partition
ENC_CHUNKS = 4
SE = S_ENC // ENC_CHUNKS  # slots per encode chunk


def _pad_groups(n):
    return -(-n // P) * P


def _chunk_group_counts(n_groups):
    """Groups per dma_gather chunk (FE edges per group). Sizes ramp up at
    the start (small first chunk -> quick desc-gen once the source prefix
    lands) and down at the end (short compute tail -> the last store lands
    early and the consumer unblocks sooner)."""
    g = P
    rem = n_groups
    tail = []
    for s in (128, 128, 256, 512):
        if rem >= s + g:
            tail.append(s)
            rem -= s
    head = []
    for s in (128, 256, 512):
        if rem >= s + g:
            head.append(s)
            rem -= s
    mid = []
    per = CHUNK // FE
    while rem > 0:
        s = min(per, rem)
        mid.append(s)
        rem -= s
    return head + mid + tail[::-1]


def plan(ptrs_list):
    """Prune the DAG bottom-up, fuse layer pairs, readiness-order stage A.

    Returns (stageA, stageB) dicts:
      n_groups: padded group count (stage A: pruned sum-layer-1 groups =
                rows of the intermediate table tA; stage B: 2048 outputs)
      n_src_rows: rows of the gathered table (A: TAB0, B: nA)
      chunks: list of (n_groups_in_chunk, src_prefix_rows)
      edge_src: per-edge source rows, production order, FE per group
    """
    p0, p1, p2, p3 = [np.asarray(p).astype(np.int64) for p in ptrs_list]
    # stage B: out group g needs L2 groups p3[2g], p3[2g+1]; each L2 group
    # h needs t-A rows p2[4h+k].
    b_l2 = p3.reshape(-1, 2)  # [2048, 2] L2 group ids
    b_src1 = p2.reshape(-1, 4)[b_l2]  # [2048, 2, 4] L1 (tA) compact... raw ids
    used1 = np.unique(b_src1)
    # stage A: one group per used L1 row; L1 row u needs L0 groups
    # p1[2u+j]; L0 group w needs t0 rows p0[4w+k].
    a_l0 = p1.reshape(-1, 2)[used1]  # [n1, 2] L0 group ids
    a_src0 = p0.reshape(-1, 4)[a_l0]  # [n1, 2, 4] t0 rows

    n1 = used1.size
    nA = _pad_groups(n1)
    srcA = np.zeros((nA, FE), dtype=np.int64)
    srcA[:n1] = a_src0.reshape(n1, FE)
    # readiness of a t0 row: encode chunk order (var slot v//128; rows 0/1
    # ready first)
    ready = np.maximum(srcA - 2, 0) // 2 // P
    order = np.argsort(ready.max(axis=1), kind="stable")
    srcA = srcA[order]
    prod = np.empty(nA, dtype=np.int64)
    prod[order] = np.arange(nA)  # compact A-group id -> production row

    relabel1 = prod[np.searchsorted(used1, b_src1)]  # [2048, 2, 4] tA rows
    srcB = relabel1.reshape(-1, FE)

    def mk(src, n_src_rows):
        n_groups = src.shape[0]
        chunks = []
        g_off = 0
        for gc in _chunk_group_counts(n_groups):
            m = int(src[g_off : g_off + gc].max()) + 1
            chunks.append((gc, m))
            g_off += gc
        return {
            "n_groups": n_groups,
            "n_src_rows": n_src_rows,
            "chunks": chunks,
            "edge_src": src.ravel(),
        }

    return [mk(srcA, TAB0), mk(srcB, nA)]


def reorder_wrap(stage):
    """Permute per-edge source ids into dma_gather order and wrap into the
    int16 [128, n_edges//16] SBUF layout (position j -> [j%16, j//16],
    replicated across the 8 gpsimd cores' 16-partition groups).

    Edge position j of chunk ci maps to partition p=j%128, slot=j//128,
    cc=slot//FE, k=slot%FE, production row = base_ci + p*csub + cc."""
    src = stage["edge_src"]
    out = np.empty(stage["n_groups"] * FE, dtype=np.int64)
    base = 0
    e_off = 0
    for gc, _m in stage["chunks"]:
        csub = gc // P
        n_e = gc * FE
        j = np.arange(n_e)
        p = j % P
        slot = j // P
        cc = slot // FE
        k = slot % FE
        row = base + p * csub + cc
        out[e_off : e_off + n_e] = src[row * FE + k]
        base += gc
        e_off += n_e
    assert out.max() < 2**15 and out.min() >= 0
    out = out.astype(np.int16)
    return np.ascontiguousarray(np.tile(out.reshape(-1, 16).T, (8, 1)))


def build_nc(meta):
    """meta: per-stage (n_groups, n_src_rows, chunks-tuple)."""
    import concourse.bacc as bacc
    import concourse.mybir as mybir
    import concourse.tile as tile

    f32 = mybir.dt.float32
    i16 = mybir.dt.int16
    Alu = mybir.AluOpType
    Act = mybir.ActivationFunctionType

    specs = [
        {"n_groups": n, "n_src_rows": s, "chunks": ch} for (n, s, ch) in meta
    ]

    nc = bacc.Bacc("TRN2", target_bir_lowering=False, debug=False)
    x = nc.dram_tensor("x", [P, S_ENC * B], f32, kind="ExternalInput")
    idx_in = [
        nc.dram_tensor(
            f"idx{l}", [P, FE * s["n_groups"] // 16], i16, kind="ExternalInput"
        )
        for l, s in enumerate(specs)
    ]
    out = nc.dram_tensor("out", [OUT_SIZES[3], B], f32, kind="ExternalOutput")

    with tile.TileContext(nc) as tc:
        with (
            tc.tile_pool(name="dram", bufs=1, space="DRAM") as dpool,
            tc.tile_pool(name="sb", bufs=4) as gp,
            tc.tile_pool(name="enc", bufs=ENC_CHUNKS) as ep,
            tc.tile_pool(name="hb", bufs=3) as hp,
            tc.tile_pool(name="tmp", bufs=2) as tp,
            tc.tile_pool(name="ix", bufs=1) as ixp,
        ):
            tables = [
                dpool.tile([s["n_src_rows"], B], f32, name=f"t{l}", tag=f"t{l}")
                for l, s in enumerate(specs)
            ]

            # Preload the combined Exp+Ln activation table once; the
            # insert_act_table_loads pass then finds every Exp/Ln already
            # covered and inserts no per-instruction reloads (1283ns each).
            ACT_SET_LN_EXP = 6  # natural_log_exp_and_others
            nc.scalar.add_instruction(
                mybir.InstLoadActFuncSet(
                    name=nc.get_next_instruction_name(),
                    ins=[],
                    outs=[],
                    act_func_set_id=ACT_SET_LN_EXP,
                )
            )

            # table0 rows 0 (-inf in the reference, never gathered) and 1
            # (zeros). Store first so the row prefix [0,2) is ready.
            z = ixp.tile([2, B], f32, tag="z")
            nc.vector.memset(z[:], 0.0)
            nc.sync.dma_start(tables[0][:][0:2, :], z[:])

            # --- encode, chunked: var v sits at partition v%128, slot
            # v//128; pos row 2+2v, neg row 3+2v.  Chunk j covers slots
            # [j*SE,(j+1)*SE) = rows [2+256*j*SE, 2+256*(j+1)*SE), a row
            # prefix, so stage-A gather chunks can start before the whole
            # encode finishes.
            xv = x[:].rearrange("p (s b) -> p s b", b=B)
            for j in range(ENC_CHUNKS):
                iv = ep.tile([P, SE, 2, B], f32, tag="enc")
                nc.sync.dma_start(iv[:][:, :, 0, :], xv[:, j * SE : (j + 1) * SE, :])
                et = hp.tile([P, SE, B], f32, tag="h")
                nc.scalar.activation(et[:], iv[:][:, :, 0, :], Act.Exp)
                nc.scalar.activation(
                    iv[:][:, :, 1, :], et[:], Act.Ln, scale=-1.0, bias=1.0
                )
                r0 = 2 + 2 * P * SE * j
                r1 = 2 + 2 * P * SE * (j + 1)
                # row = r0 + 256*s + 2*p + k
                nc.sync.dma_start(
                    tables[0][:][r0:r1, :].rearrange("(s p k) b -> p s k b", p=P, k=2),
                    iv[:],
                )

            # --- index list loads ---
            ix_t = []
            for l, s in enumerate(specs):
                t = ixp.tile([P, FE * s["n_groups"] // 16], i16, tag=f"ix{l}")
                nc.sync.dma_start(t[:], idx_in[l][:])
                ix_t.append(t)

            # --- fused gather + product-sum + logsumexp stages ---
            for l, s in enumerate(specs):
                dst_tile = tables[l + 1][:] if l + 1 < len(specs) else out[:]
                g_off = 0
                e_off = 0
                for gc, m_src in s["chunks"]:
                    csub = gc // P
                    ch = gc * FE
                    S = ch // P
                    g = gp.tile([P, S, B], f32, tag="g")
                    nc.gpsimd.dma_gather(
                        g[:],
                        tables[l][:][0:m_src, :],
                        ix_t[l][:, e_off // 16 : (e_off + ch) // 16],
                        ch,
                        ch,
                        B,
                        single_packet=False,
                    )
                    # [p, group, pair(2), fanin(4), b]
                    v = g[:].rearrange("p (c j k) b -> p c j k b", j=2, k=4)
                    s01 = tp.tile([P, csub, 2, B], f32, tag="m")
                    s23 = tp.tile([P, csub, 2, B], f32, tag="n")
                    ss = gp.tile([P, csub, 2, B], f32, tag="s")
                    nc.vector.tensor_add(s01[:], v[:, :, :, 0, :], v[:, :, :, 1, :])
                    nc.vector.tensor_add(s23[:], v[:, :, :, 2, :], v[:, :, :, 3, :])
                    nc.vector.tensor_add(ss[:], s01[:], s23[:])
                    a = ss[:][:, :, 0, :]
                    b = ss[:][:, :, 1, :]
                    h = hp.tile([P, csub, B], f32, tag="h")
                    if l == 0:
                        # logsumexp(a,b) = a + ln(1+exp(b-a)); |b-a| < 40
                        # here so exp stays in f32 range.
                        d = tp.tile([P, csub, B], f32, tag="d")
                        sp = tp.tile([P, csub, B], f32, tag="sp")
                        nc.vector.tensor_tensor(d[:], b, a, op=Alu.subtract)
                        nc.scalar.activation(d[:], d[:], Act.Exp)
                        nc.scalar.activation(sp[:], d[:], Act.Ln, bias=1.0)
                        nc.vector.tensor_add(h[:], a, sp[:])
                    else:
                        # wider range: logsumexp = max + ln(1+exp(min-max))
                        m = tp.tile([P, csub, B], f32, tag="d")
                        mn = tp.tile([P, csub, B], f32, tag="sp")
                        sp = tp.tile([P, csub, B], f32, tag="sq")
                        nc.vector.tensor_tensor(m[:], a, b, op=Alu.max)
                        nc.vector.tensor_tensor(mn[:], a, b, op=Alu.min)
                        nc.vector.tensor_tensor(mn[:], mn[:], m[:], op=Alu.subtract)
                        nc.scalar.activation(mn[:], mn[:], Act.Exp)
                        nc.scalar.activation(sp[:], mn[:], Act.Ln, bias=1.0)
                        nc.vector.tensor_add(h[:], m[:], sp[:])
                    # chunk produces rows [g_off, g_off + P*csub):
                    # row = g_off + p*csub + cc
                    nc.sync.dma_start(
                        dst_tile[g_off : g_off + P * csub, :].rearrange(
                            "(p c) b -> p (c b)", p=P
                        ),
                        h[:].rearrange("p c b -> p (c b)"),
                    )
                    g_off += P * csub
                    e_off += ch
    nc.compile()
    return nc


def host_prep(x, ptrs_list, seg_list):
    """Host-side sharding + pruning + index preprocessing -> per-core maps."""
    x = np.asarray(x, dtype=np.float32)
    for l, (n_out, f) in enumerate(zip(OUT_SIZES, FANINS)):
        seg = np.asarray(seg_list[l]).astype(np.int64)
        expected = np.repeat(np.arange(n_out, dtype=np.int64), f)
        assert np.array_equal(seg, expected), f"layer {l}: non-uniform segments"

    stages = plan(ptrs_list)
    idx_maps = {f"idx{l}": reorder_wrap(s) for l, s in enumerate(stages)}

    batch = x.shape[1]
    bpc = batch // NCORES
    in_maps = []
    for i in range(NCORES):
        xs = x[:, i * bpc : (i + 1) * bpc]
        # partition p, slot s holds var s*128+p (slot-major var layout)
        xv = np.ascontiguousarray(
            xs.reshape(S_ENC, P, bpc).transpose(1, 0, 2)
        ).reshape(P, -1)
        in_maps.append({"x": xv, **idx_maps})
    return in_maps


def _meta(stages):
    return tuple(
        (s["n_groups"], s["n_src_rows"], tuple(s["chunks"])) for s in stages
    )


_CACHE = {}


def _get_nc(meta=None):
    if meta is None:
        meta = _CACHE.get("meta")
        assert meta is not None, "call kernel() first"
    if _CACHE.get("meta") != meta:
        _CACHE["nc"] = build_nc(meta)
        _CACHE["meta"] = meta
    return _CACHE["nc"]


def kernel(x, ptrs0, seg0, ptrs1, seg1, ptrs2, seg2, ptrs3, seg3):
    from concourse.bass_utils import run_bass_kernel_spmd

    ptrs_list = [ptrs0, ptrs1, ptrs2, ptrs3]
    stages = plan(ptrs_list)
    nc = _get_nc(_meta(stages))
    in_maps = host_prep(x, ptrs_list, [seg0, seg1, seg2, seg3])
    res = run_bass_kernel_spmd(nc, in_maps, core_ids=list(range(NCORES)))
    outs = [r["out"] for r in res.results]
    return np.concatenate(outs, axis=1)


# revision 11
# speedup vs baseline: 1.9255x; 1.0293x over previous
"""Trainium2 Bass kernel for the sum-product "knowledge layer" network.

Computation (see problem reference):
  h0 = encode(x): 8194-row table [-inf, 0, pos0, neg0, pos1, neg1, ...]
       with pos = x (log-probs), neg = log(1 - exp(x)), per batch column.
  4 alternating layers, each: gather rows by ptrs, then segment-reduce over
  contiguous fanin groups (fanin 4 sum-of-logs "product" layers, fanin 2
  logsumexp "sum" layers).

Strategy (pure batch data-parallelism, 8 NeuronCores):
  - Shard the 512 batch columns 8 ways -> 64 columns per core.
  - Per core every tensor lives in DRAM as [rows, 64] fp32; one row = 256B.
  - Gathers use the SWDGE dma_gather instruction: int16 index list in SBUF,
    each index pulls one 256B row from the DRAM table; index list position j
    lands at SBUF partition j%128, free slot j//128.
  - DAG pruning (host side, per input set): working back from the 2048
    output rows, only rows actually referenced downstream are computed.
  - Layer fusion: layers 0+1 fuse into stage A, layers 2+3 into stage B.
    A stage group gathers its 8 leaf rows (2 product groups x fanin 4),
    sums each quad on DVE, then logsumexps the pair - so the intermediate
    product table never exists in DRAM, removing its store and a pipeline
    boundary, at the cost of recomputing product rows referenced by more
    than one sum edge (~4% more gather descriptors).
  - Cross-layer software pipelining: stage A's output rows are produced in
    chunk order (chunk ci stores rows [base, base + P*csub), row = base +
    p*csub + cc), A groups are sorted by the max table-0 row they
    reference, and every gather chunk's source AP is narrowed to the exact
    table prefix it needs, so the tile framework only serializes a gather
    against the stores that overlap its prefix. The encode is likewise
    chunked (vars are laid out slot-major: var v lives at partition v%128,
    slot v//128, so encode chunk j fills a table-0 row prefix). Chunk
    sizes ramp up then down so desc-gen and compute tails stay short.
  - Sum reduction: logsumexp(a,b) = a + ln(1+exp(b-a)) in stage A (|b-a|
    < 40, f32-exp-safe) and max + ln(1+exp(min-max)) in stage B (wider
    range), with the Exp+Ln activation table preloaded once (set 6) so
    the compiler inserts no per-instruction table reloads.
"""

import numpy as np

P = 128
B = 64  # batch columns per core
NCORES = 8
N_VARS = 4096
BATCH = 512
TAB0 = 2 * N_VARS + 2  # 8194
OUT_SIZES = [16384, 8192, 4096, 2048]
FANINS = [4, 2, 4, 2]
FE = 8  # edges per fused group: 2 (sum fanin) x 4 (product fanin)
CHUNK = 8192  # max gather indices per dma_gather instruction
S_ENC = N_VARS // P  # 32 encode slots per partition
ENC_CHUNKS = 4
SE = S_ENC // ENC_CHUNKS  # slots per encode chunk


def _pad_groups(n):
    return -(-n // P) * P


def _chunk_group_counts(n_groups):
    """Groups per dma_gather chunk (FE edges per group). Sizes ramp up at
    the start (small first chunk -> quick desc-gen once the source prefix
    lands) and down at the end (short compute tail -> the last store lands
    early and the consumer unblocks sooner)."""
    g = P
    rem = n_groups
    tail = []
    for s in (128, 128, 256, 512):
        if rem >= s + g:
            tail.append(s)
            rem -= s
    head = []
    for s in (128, 256, 512):
        if rem >= s + g:
            head.append(s)
            rem -= s
    mid = []
    per = CHUNK // FE
    while rem > 0:
        s = min(per, rem)
        mid.append(s)
        rem -= s
    return head + mid + tail[::-1]


def plan(ptrs_list):
    """Prune the DAG bottom-up, fuse layer pairs, readiness-order stage A.

    Returns (stageA, stageB) dicts:
      n_groups: padded group count (stage A: pruned sum-layer-1 groups =
                rows of the intermediate table tA; stage B: 2048 outputs)
      n_src_rows: rows of the gathered table (A: TAB0, B: nA)
      chunks: list of (n_groups_in_chunk, src_prefix_rows)
      edge_src: per-edge source rows, production order, FE per group
    """
    p0, p1, p2, p3 = [np.asarray(p).astype(np.int64) for p in ptrs_list]
    # stage B: out group g needs L2 groups p3[2g], p3[2g+1]; each L2 group
    # h needs t-A rows p2[4h+k].
    b_l2 = p3.reshape(-1, 2)  # [2048, 2] L2 group ids
    b_src1 = p2.reshape(-1, 4)[b_l2]  # [2048, 2, 4] L1 (tA) compact... raw ids
    used1 = np.unique(b_src1)
    # stage A: one group per used L1 row; L1 row u needs L0 groups
    # p1[2u+j]; L0 group w needs t0 rows p0[4w+k].
    a_l0 = p1.reshape(-1, 2)[used1]  # [n1, 2] L0 group ids
    a_src0 = p0.reshape(-1, 4)[a_l0]  # [n1, 2, 4] t0 rows

    n1 = used1.size
    nA = _pad_groups(n1)
    srcA = np.zeros((nA, FE), dtype=np.int64)
    srcA[:n1] = a_src0.reshape(n1, FE)
    # readiness of a t0 row: encode chunk order (var slot v//128; rows 0/1
    # ready first)
    ready = np.maximum(srcA - 2, 0) // 2 // P
    order = np.argsort(ready.max(axis=1), kind="stable")
    srcA = srcA[order]
    prod = np.empty(nA, dtype=np.int64)
    prod[order] = np.arange(nA)  # compact A-group id -> production row

    relabel1 = prod[np.searchsorted(used1, b_src1)]  # [2048, 2, 4] tA rows
    srcB = relabel1.reshape(-1, FE)
    # Stage B output rows need no fixed order either - the host unpermutes
    # rows after the run - so readiness-sort B too: its early chunks then
    # only need a tA prefix and can overlap stage A's tail.
    orderB = np.argsort(srcB.max(axis=1), kind="stable")
    srcB = srcB[orderB]
    prodB = np.empty(srcB.shape[0], dtype=np.int64)
    prodB[orderB] = np.arange(srcB.shape[0])  # out group g -> produced row

    def mk(src, n_src_rows):
        n_groups = src.shape[0]
        chunks = []
        g_off = 0
        for gc in _chunk_group_counts(n_groups):
            m = int(src[g_off : g_off + gc].max()) + 1
            chunks.append((gc, m))
            g_off += gc
        return {
            "n_groups": n_groups,
            "n_src_rows": n_src_rows,
            "chunks": chunks,
            "edge_src": src.ravel(),
        }

    stages = [mk(srcA, TAB0), mk(srcB, nA)]
    stages[1]["out_perm"] = prodB
    return stages


def reorder_wrap(stage):
    """Permute per-edge source ids into dma_gather order and wrap into the
    int16 [128, n_edges//16] SBUF layout (position j -> [j%16, j//16],
    replicated across the 8 gpsimd cores' 16-partition groups).

    Edge position j of chunk ci maps to partition p=j%128, slot=j//128,
    cc=slot//FE, k=slot%FE, production row = base_ci + p*csub + cc."""
    src = stage["edge_src"]
    out = np.empty(stage["n_groups"] * FE, dtype=np.int64)
    base = 0
    e_off = 0
    for gc, _m in stage["chunks"]:
        csub = gc // P
        n_e = gc * FE
        j = np.arange(n_e)
        p = j % P
        slot = j // P
        cc = slot // FE
        k = slot % FE
        row = base + p * csub + cc
        out[e_off : e_off + n_e] = src[row * FE + k]
        base += gc
        e_off += n_e
    assert out.max() < 2**15 and out.min() >= 0
    out = out.astype(np.int16)
    return np.ascontiguousarray(np.tile(out.reshape(-1, 16).T, (8, 1)))


def build_nc(meta):
    """meta: per-stage (n_groups, n_src_rows, chunks-tuple)."""
    import concourse.bacc as bacc
    import concourse.mybir as mybir
    import concourse.tile as tile

    f32 = mybir.dt.float32
    i16 = mybir.dt.int16
    Alu = mybir.AluOpType
    Act = mybir.ActivationFunctionType

    specs = [
        {"n_groups": n, "n_src_rows": s, "chunks": ch} for (n, s, ch) in meta
    ]

    nc = bacc.Bacc("TRN2", target_bir_lowering=False, debug=False)
    x = nc.dram_tensor("x", [P, S_ENC * B], f32, kind="ExternalInput")
    idx_in = [
        nc.dram_tensor(
            f"idx{l}", [P, FE * s["n_groups"] // 16], i16, kind="ExternalInput"
        )
        for l, s in enumerate(specs)
    ]
    out = nc.dram_tensor("out", [OUT_SIZES[3], B], f32, kind="ExternalOutput")

    with tile.TileContext(nc) as tc:
        with (
            tc.tile_pool(name="dram", bufs=1, space="DRAM") as dpool,
            tc.tile_pool(name="sb", bufs=4) as gp,
            tc.tile_pool(name="enc", bufs=ENC_CHUNKS) as ep,
            tc.tile_pool(name="hb", bufs=3) as hp,
            tc.tile_pool(name="tmp", bufs=2) as tp,
            tc.tile_pool(name="ix", bufs=1) as ixp,
        ):
            tables = [
                dpool.tile([s["n_src_rows"], B], f32, name=f"t{l}", tag=f"t{l}")
                for l, s in enumerate(specs)
            ]

            # Preload the combined Exp+Ln activation table once; the
            # insert_act_table_loads pass then finds every Exp/Ln already
            # covered and inserts no per-instruction reloads (1283ns each).
            ACT_SET_LN_EXP = 6  # natural_log_exp_and_others
            nc.scalar.add_instruction(
                mybir.InstLoadActFuncSet(
                    name=nc.get_next_instruction_name(),
                    ins=[],
                    outs=[],
                    act_func_set_id=ACT_SET_LN_EXP,
                )
            )

            # table0 rows 0 (-inf in the reference, never gathered) and 1
            # (zeros). Store first so the row prefix [0,2) is ready.
            z = ixp.tile([2, B], f32, tag="z")
            nc.vector.memset(z[:], 0.0)
            nc.sync.dma_start(tables[0][:][0:2, :], z[:])

            # --- encode, chunked: var v sits at partition v%128, slot
            # v//128; pos row 2+2v, neg row 3+2v.  Chunk j covers slots
            # [j*SE,(j+1)*SE) = rows [2+256*j*SE, 2+256*(j+1)*SE), a row
            # prefix, so stage-A gather chunks can start before the whole
            # encode finishes.
            xv = x[:].rearrange("p (s b) -> p s b", b=B)
            for j in range(ENC_CHUNKS):
                iv = ep.tile([P, SE, 2, B], f32, tag="enc")
                nc.sync.dma_start(iv[:][:, :, 0, :], xv[:, j * SE : (j + 1) * SE, :])
                et = hp.tile([P, SE, B], f32, tag="h")
                nc.scalar.activation(et[:], iv[:][:, :, 0, :], Act.Exp)
                nc.scalar.activation(
                    iv[:][:, :, 1, :], et[:], Act.Ln, scale=-1.0, bias=1.0
                )
                r0 = 2 + 2 * P * SE * j
                r1 = 2 + 2 * P * SE * (j + 1)
                # row = r0 + 256*s + 2*p + k
                nc.sync.dma_start(
                    tables[0][:][r0:r1, :].rearrange("(s p k) b -> p s k b", p=P, k=2),
                    iv[:],
                )

            # --- index list loads ---
            ix_t = []
            for l, s in enumerate(specs):
                t = ixp.tile([P, FE * s["n_groups"] // 16], i16, tag=f"ix{l}")
                nc.sync.dma_start(t[:], idx_in[l][:])
                ix_t.append(t)

            # --- fused gather + product-sum + logsumexp stages ---
            for l, s in enumerate(specs):
                dst_tile = tables[l + 1][:] if l + 1 < len(specs) else out[:]
                g_off = 0
                e_off = 0
                for gc, m_src in s["chunks"]:
                    csub = gc // P
                    ch = gc * FE
                    S = ch // P
                    g = gp.tile([P, S, B], f32, tag="g")
                    nc.gpsimd.dma_gather(
                        g[:],
                        tables[l][:][0:m_src, :],
                        ix_t[l][:, e_off // 16 : (e_off + ch) // 16],
                        ch,
                        ch,
                        B,
                        single_packet=False,
                    )
                    # [p, group, pair(2), fanin(4), b]
                    v = g[:].rearrange("p (c j k) b -> p c j k b", j=2, k=4)
                    s01 = tp.tile([P, csub, 2, B], f32, tag="m")
                    s23 = tp.tile([P, csub, 2, B], f32, tag="n")
                    ss = gp.tile([P, csub, 2, B], f32, tag="s")
                    nc.vector.tensor_add(s01[:], v[:, :, :, 0, :], v[:, :, :, 1, :])
                    nc.vector.tensor_add(s23[:], v[:, :, :, 2, :], v[:, :, :, 3, :])
                    nc.vector.tensor_add(ss[:], s01[:], s23[:])
                    a = ss[:][:, :, 0, :]
                    b = ss[:][:, :, 1, :]
                    h = hp.tile([P, csub, B], f32, tag="h")
                    if l == 0:
                        # logsumexp(a,b) = a + ln(1+exp(b-a)); |b-a| < 40
                        # here so exp stays in f32 range.
                        d = tp.tile([P, csub, B], f32, tag="d")
                        sp = tp.tile([P, csub, B], f32, tag="sp")
                        nc.vector.tensor_tensor(d[:], b, a, op=Alu.subtract)
                        nc.scalar.activation(d[:], d[:], Act.Exp)
                        nc.scalar.activation(sp[:], d[:], Act.Ln, bias=1.0)
                        nc.vector.tensor_add(h[:], a, sp[:])
                    else:
                        # wider range: logsumexp = max + ln(1+exp(min-max))
                        m = tp.tile([P, csub, B], f32, tag="d")
                        mn = tp.tile([P, csub, B], f32, tag="sp")
                        sp = tp.tile([P, csub, B], f32, tag="sq")
                        nc.vector.tensor_tensor(m[:], a, b, op=Alu.max)
                        nc.vector.tensor_tensor(mn[:], a, b, op=Alu.min)
                        nc.vector.tensor_tensor(mn[:], mn[:], m[:], op=Alu.subtract)
                        nc.scalar.activation(mn[:], mn[:], Act.Exp)
                        nc.scalar.activation(sp[:], mn[:], Act.Ln, bias=1.0)
                        nc.vector.tensor_add(h[:], m[:], sp[:])
                    # chunk produces rows [g_off, g_off + P*csub):
                    # row = g_off + p*csub + cc
                    nc.sync.dma_start(
                        dst_tile[g_off : g_off + P * csub, :].rearrange(
                            "(p c) b -> p (c b)", p=P
                        ),
                        h[:].rearrange("p c b -> p (c b)"),
                    )
                    g_off += P * csub
                    e_off += ch
    nc.compile()
    return nc


def host_prep(x, ptrs_list, seg_list):
    """Host-side sharding + pruning + index preprocessing -> per-core maps."""
    x = np.asarray(x, dtype=np.float32)
    for l, (n_out, f) in enumerate(zip(OUT_SIZES, FANINS)):
        seg = np.asarray(seg_list[l]).astype(np.int64)
        expected = np.repeat(np.arange(n_out, dtype=np.int64), f)
        assert np.array_equal(seg, expected), f"layer {l}: non-uniform segments"

    stages = plan(ptrs_list)
    idx_maps = {f"idx{l}": reorder_wrap(s) for l, s in enumerate(stages)}

    batch = x.shape[1]
    bpc = batch // NCORES
    in_maps = []
    for i in range(NCORES):
        xs = x[:, i * bpc : (i + 1) * bpc]
        # partition p, slot s holds var s*128+p (slot-major var layout)
        xv = np.ascontiguousarray(
            xs.reshape(S_ENC, P, bpc).transpose(1, 0, 2)
        ).reshape(P, -1)
        in_maps.append({"x": xv, **idx_maps})
    return in_maps


def _meta(stages):
    return tuple(
        (s["n_groups"], s["n_src_rows"], tuple(s["chunks"])) for s in stages
    )


_CACHE = {}


def _get_nc(meta=None):
    if meta is None:
        meta = _CACHE.get("meta")
        assert meta is not None, "call kernel() first"
    if _CACHE.get("meta") != meta:
        _CACHE["nc"] = build_nc(meta)
        _CACHE["meta"] = meta
    return _CACHE["nc"]


def kernel(x, ptrs0, seg0, ptrs1, seg1, ptrs2, seg2, ptrs3, seg3):
    from concourse.bass_utils import run_bass_kernel_spmd

    ptrs_list = [ptrs0, ptrs1, ptrs2, ptrs3]
    stages = plan(ptrs_list)
    nc = _get_nc(_meta(stages))
    in_maps = host_prep(x, ptrs_list, [seg0, seg1, seg2, seg3])
    res = run_bass_kernel_spmd(nc, in_maps, core_ids=list(range(NCORES)))
    outs = [r["out"] for r in res.results]
    full = np.concatenate(outs, axis=1)
    # rows were produced in readiness order; map back to natural order
    return np.ascontiguousarray(full[stages[1]["out_perm"]])


# revision 13
# speedup vs baseline: 1.9618x; 1.0189x over previous
"""Trainium2 Bass kernel for the sum-product "knowledge layer" network.

Computation (see problem reference):
  h0 = encode(x): 8194-row table [-inf, 0, pos0, neg0, pos1, neg1, ...]
       with pos = x (log-probs), neg = log(1 - exp(x)), per batch column.
  4 alternating layers, each: gather rows by ptrs, then segment-reduce over
  contiguous fanin groups (fanin 4 sum-of-logs "product" layers, fanin 2
  logsumexp "sum" layers).

Strategy (pure batch data-parallelism, 8 NeuronCores):
  - Shard the 512 batch columns 8 ways -> 64 columns per core.
  - Per core every tensor lives in DRAM as [rows, 64] fp32; one row = 256B.
  - Gathers use the SWDGE dma_gather instruction: int16 index list in SBUF,
    each index pulls one 256B row from the DRAM table; index list position j
    lands at SBUF partition j%128, free slot j//128.
  - DAG pruning (host side, per input set): working back from the 2048
    output rows, only rows actually referenced downstream are computed.
  - Layer fusion: layers 0+1 fuse into stage A, layers 2+3 into stage B.
    A stage group gathers its 8 leaf rows (2 product groups x fanin 4),
    sums each quad on DVE, then logsumexps the pair - so the intermediate
    product table never exists in DRAM, removing its store and a pipeline
    boundary, at the cost of recomputing product rows referenced by more
    than one sum edge (~4% more gather descriptors).
  - Cross-layer software pipelining: stage A's output rows are produced in
    chunk order (chunk ci stores rows [base, base + P*csub), row = base +
    p*csub + cc), A groups are sorted by the max table-0 row they
    reference, and every gather chunk's source AP is narrowed to the exact
    table prefix it needs, so the tile framework only serializes a gather
    against the stores that overlap its prefix. The encode is likewise
    chunked (vars are laid out slot-major: var v lives at partition v%128,
    slot v//128, so encode chunk j fills a table-0 row prefix). Chunk
    sizes ramp up then down so desc-gen and compute tails stay short.
  - Sum reduction: logsumexp(a,b) = a + ln(1+exp(b-a)) in stage A (|b-a|
    < 40, f32-exp-safe) and max + ln(1+exp(min-max)) in stage B (wider
    range), with the Exp+Ln activation table preloaded once (set 6) so
    the compiler inserts no per-instruction table reloads.
"""

import numpy as np

P = 128
B = 64  # batch columns per core
NCORES = 8
N_VARS = 4096
BATCH = 512
TAB0 = 2 * N_VARS + 2  # 8194
OUT_SIZES = [16384, 8192, 4096, 2048]
FANINS = [4, 2, 4, 2]
FE = 8  # edges per fused group: 2 (sum fanin) x 4 (product fanin)
CHUNK = 8192  # max gather indices per dma_gather instruction
S_ENC = N_VARS // P  # 32 encode slots per partition
ENC_CHUNKS = 4
SE = S_ENC // ENC_CHUNKS  # slots per encode chunk


def _pad_groups(n):
    return -(-n // P) * P


def _chunk_group_counts(n_groups):
    """Groups per dma_gather chunk (FE edges per group). Sizes ramp up at
    the start (small first chunk -> quick desc-gen once the source prefix
    lands) and down at the end (short compute tail -> the last store lands
    early and the consumer unblocks sooner)."""
    g = P
    rem = n_groups
    tail = []
    for s in (128, 128, 256, 512):
        if rem >= s + g:
            tail.append(s)
            rem -= s
    head = []
    for s in (128, 256, 512):
        if rem >= s + g:
            head.append(s)
            rem -= s
    mid = []
    per = CHUNK // FE
    while rem > 0:
        s = min(per, rem)
        mid.append(s)
        rem -= s
    return head + mid + tail[::-1]


def plan(ptrs_list):
    """Prune the DAG bottom-up, fuse layer pairs, readiness-order stage A.

    Returns (stageA, stageB) dicts:
      n_groups: padded group count (stage A: pruned sum-layer-1 groups =
                rows of the intermediate table tA; stage B: 2048 outputs)
      n_src_rows: rows of the gathered table (A: TAB0, B: nA)
      chunks: list of (n_groups_in_chunk, src_prefix_rows)
      edge_src: per-edge source rows, production order, FE per group
    """
    p0, p1, p2, p3 = [np.asarray(p).astype(np.int64) for p in ptrs_list]
    # stage B: out group g needs L2 groups p3[2g], p3[2g+1]; each L2 group
    # h needs t-A rows p2[4h+k].
    b_l2 = p3.reshape(-1, 2)  # [2048, 2] L2 group ids
    b_src1 = p2.reshape(-1, 4)[b_l2]  # [2048, 2, 4] L1 (tA) compact... raw ids
    used1 = np.unique(b_src1)
    # stage A: one group per used L1 row; L1 row u needs L0 groups
    # p1[2u+j]; L0 group w needs t0 rows p0[4w+k].
    a_l0 = p1.reshape(-1, 2)[used1]  # [n1, 2] L0 group ids
    a_src0 = p0.reshape(-1, 4)[a_l0]  # [n1, 2, 4] t0 rows

    n1 = used1.size
    nA = _pad_groups(n1)
    srcA = np.zeros((nA, FE), dtype=np.int64)
    srcA[:n1] = a_src0.reshape(n1, FE)
    # readiness of a t0 row: encode chunk order (var slot v//128; rows 0/1
    # ready first)
    ready = np.maximum(srcA - 2, 0) // 2 // P
    order = np.argsort(ready.max(axis=1), kind="stable")
    srcA = srcA[order]
    prod = np.empty(nA, dtype=np.int64)
    prod[order] = np.arange(nA)  # compact A-group id -> production row

    relabel1 = prod[np.searchsorted(used1, b_src1)]  # [2048, 2, 4] tA rows
    srcB = relabel1.reshape(-1, FE)
    # Stage B output rows need no fixed order either - the host unpermutes
    # rows after the run - so readiness-sort B too: its early chunks then
    # only need a tA prefix and can overlap stage A's tail.
    orderB = np.argsort(srcB.max(axis=1), kind="stable")
    srcB = srcB[orderB]
    prodB = np.empty(srcB.shape[0], dtype=np.int64)
    prodB[orderB] = np.arange(srcB.shape[0])  # out group g -> produced row

    def mk(src, n_src_rows):
        n_groups = src.shape[0]
        chunks = []
        g_off = 0
        for gc in _chunk_group_counts(n_groups):
            m = int(src[g_off : g_off + gc].max()) + 1
            chunks.append((gc, m))
            g_off += gc
        return {
            "n_groups": n_groups,
            "n_src_rows": n_src_rows,
            "chunks": chunks,
            "edge_src": src.ravel(),
        }

    stages = [mk(srcA, TAB0), mk(srcB, nA)]
    stages[1]["out_perm"] = prodB
    return stages


def reorder_wrap(stage):
    """Permute per-edge source ids into dma_gather order and wrap into the
    int16 [128, n_edges//16] SBUF layout (position j -> [j%16, j//16],
    replicated across the 8 gpsimd cores' 16-partition groups).

    Edge position j of chunk ci maps to partition p=j%128, slot=j//128,
    cc=slot//FE, k=slot%FE, production row = base_ci + p*csub + cc."""
    src = stage["edge_src"]
    out = np.empty(stage["n_groups"] * FE, dtype=np.int64)
    base = 0
    e_off = 0
    for gc, _m in stage["chunks"]:
        csub = gc // P
        n_e = gc * FE
        j = np.arange(n_e)
        p = j % P
        slot = j // P
        cc = slot // FE
        k = slot % FE
        row = base + p * csub + cc
        out[e_off : e_off + n_e] = src[row * FE + k]
        base += gc
        e_off += n_e
    assert out.max() < 2**15 and out.min() >= 0
    out = out.astype(np.int16)
    return np.ascontiguousarray(np.tile(out.reshape(-1, 16).T, (8, 1)))


def build_nc(meta):
    """meta: per-stage (n_groups, n_src_rows, chunks-tuple)."""
    import concourse.bacc as bacc
    import concourse.mybir as mybir
    import concourse.tile as tile

    f32 = mybir.dt.float32
    i16 = mybir.dt.int16
    Alu = mybir.AluOpType
    Act = mybir.ActivationFunctionType

    specs = [
        {"n_groups": n, "n_src_rows": s, "chunks": ch} for (n, s, ch) in meta
    ]

    nc = bacc.Bacc("TRN2", target_bir_lowering=False, debug=False)
    x = nc.dram_tensor("x", [P, S_ENC * B], f32, kind="ExternalInput")
    idx_in = [
        nc.dram_tensor(
            f"idx{l}", [P, FE * s["n_groups"] // 16], i16, kind="ExternalInput"
        )
        for l, s in enumerate(specs)
    ]
    out = nc.dram_tensor("out", [OUT_SIZES[3], B], f32, kind="ExternalOutput")

    with tile.TileContext(nc) as tc:
        with (
            tc.tile_pool(name="dram", bufs=1, space="DRAM") as dpool,
            tc.tile_pool(name="sb", bufs=4) as gp,
            tc.tile_pool(name="enc", bufs=ENC_CHUNKS) as ep,
            tc.tile_pool(name="hb", bufs=3) as hp,
            tc.tile_pool(name="tmp", bufs=2) as tp,
            tc.tile_pool(name="ix", bufs=1) as ixp,
        ):
            tables = [
                dpool.tile([s["n_src_rows"], B], f32, name=f"t{l}", tag=f"t{l}")
                for l, s in enumerate(specs)
            ]

            # Preload the combined Exp+Ln activation table once; the
            # insert_act_table_loads pass then finds every Exp/Ln already
            # covered and inserts no per-instruction reloads (1283ns each).
            ACT_SET_LN_EXP = 6  # natural_log_exp_and_others
            nc.scalar.add_instruction(
                mybir.InstLoadActFuncSet(
                    name=nc.get_next_instruction_name(),
                    ins=[],
                    outs=[],
                    act_func_set_id=ACT_SET_LN_EXP,
                )
            )

            # table0 rows 0 (-inf in the reference, never gathered) and 1
            # (zeros). Store first so the row prefix [0,2) is ready.
            z = ixp.tile([2, B], f32, tag="z")
            nc.vector.memset(z[:], 0.0)
            nc.sync.dma_start(tables[0][:][0:2, :], z[:])

            # --- encode, chunked: var v sits at partition v%128, slot
            # v//128; pos row 2+2v, neg row 3+2v.  Chunk j covers slots
            # [j*SE,(j+1)*SE) = rows [2+256*j*SE, 2+256*(j+1)*SE), a row
            # prefix, so stage-A gather chunks can start before the whole
            # encode finishes.
            # All independent loads are emitted before any store so the
            # in-order DMA queue never has a compute-gated store blocking a
            # ready load: x chunks first, then the index lists.
            xv = x[:].rearrange("p (s b) -> p s b", b=B)
            xls = []
            for j in range(ENC_CHUNKS):
                # contiguous destination: 2KB DMA descriptors (the
                # interleaved iv[:, :, 0, :] view would halve them to 256B
                # and pay the sub-512B 2x transfer penalty)
                xl = ep.tile([P, SE, B], f32, tag="xl")
                nc.sync.dma_start(xl[:], xv[:, j * SE : (j + 1) * SE, :])
                xls.append(xl)
            ix_t = []
            for l, s in enumerate(specs):
                t = ixp.tile([P, FE * s["n_groups"] // 16], i16, tag=f"ix{l}")
                nc.sync.dma_start(t[:], idx_in[l][:])
                ix_t.append(t)
            for j in range(ENC_CHUNKS):
                xl = xls[j]
                iv = ep.tile([P, SE, 2, B], f32, tag="enc")
                et = hp.tile([P, SE, B], f32, tag="h")
                nc.scalar.copy(iv[:][:, :, 0, :], xl[:])
                nc.scalar.activation(et[:], xl[:], Act.Exp)
                nc.scalar.activation(
                    iv[:][:, :, 1, :], et[:], Act.Ln, scale=-1.0, bias=1.0
                )
                r0 = 2 + 2 * P * SE * j
                r1 = 2 + 2 * P * SE * (j + 1)
                # row = r0 + 256*s + 2*p + k
                nc.sync.dma_start(
                    tables[0][:][r0:r1, :].rearrange("(s p k) b -> p s k b", p=P, k=2),
                    iv[:],
                )

            # --- fused gather + product-sum + logsumexp stages ---
            for l, s in enumerate(specs):
                dst_tile = tables[l + 1][:] if l + 1 < len(specs) else out[:]
                g_off = 0
                e_off = 0
                for gc, m_src in s["chunks"]:
                    csub = gc // P
                    ch = gc * FE
                    S = ch // P
                    g = gp.tile([P, S, B], f32, tag="g")
                    nc.gpsimd.dma_gather(
                        g[:],
                        tables[l][:][0:m_src, :],
                        ix_t[l][:, e_off // 16 : (e_off + ch) // 16],
                        ch,
                        ch,
                        B,
                        single_packet=False,
                    )
                    # [p, group, pair(2), fanin(4), b]
                    v = g[:].rearrange("p (c j k) b -> p c j k b", j=2, k=4)
                    s01 = tp.tile([P, csub, 2, B], f32, tag="m")
                    s23 = tp.tile([P, csub, 2, B], f32, tag="n")
                    ss = gp.tile([P, csub, 2, B], f32, tag="s")
                    nc.vector.tensor_add(s01[:], v[:, :, :, 0, :], v[:, :, :, 1, :])
                    nc.vector.tensor_add(s23[:], v[:, :, :, 2, :], v[:, :, :, 3, :])
                    nc.vector.tensor_add(ss[:], s01[:], s23[:])
                    a = ss[:][:, :, 0, :]
                    b = ss[:][:, :, 1, :]
                    h = hp.tile([P, csub, B], f32, tag="h")
                    if l == 0:
                        # logsumexp(a,b) = a + ln(1+exp(b-a)); |b-a| < 40
                        # here so exp stays in f32 range.
                        d = tp.tile([P, csub, B], f32, tag="d")
                        sp = tp.tile([P, csub, B], f32, tag="sp")
                        nc.vector.tensor_tensor(d[:], b, a, op=Alu.subtract)
                        nc.scalar.activation(d[:], d[:], Act.Exp)
                        nc.scalar.activation(sp[:], d[:], Act.Ln, bias=1.0)
                        nc.vector.tensor_add(h[:], a, sp[:])
                    else:
                        # wider range: logsumexp = max + ln(1+exp(min-max))
                        m = tp.tile([P, csub, B], f32, tag="d")
                        mn = tp.tile([P, csub, B], f32, tag="sp")
                        sp = tp.tile([P, csub, B], f32, tag="sq")
                        nc.vector.tensor_tensor(m[:], a, b, op=Alu.max)
                        nc.vector.tensor_tensor(mn[:], a, b, op=Alu.min)
                        nc.vector.tensor_tensor(mn[:], mn[:], m[:], op=Alu.subtract)
                        nc.scalar.activation(mn[:], mn[:], Act.Exp)
                        nc.scalar.activation(sp[:], mn[:], Act.Ln, bias=1.0)
                        nc.vector.tensor_add(h[:], m[:], sp[:])
                    # chunk produces rows [g_off, g_off + P*csub):
                    # row = g_off + p*csub + cc
                    nc.sync.dma_start(
                        dst_tile[g_off : g_off + P * csub, :].rearrange(
                            "(p c) b -> p (c b)", p=P
                        ),
                        h[:].rearrange("p c b -> p (c b)"),
                    )
                    g_off += P * csub
                    e_off += ch
    nc.compile()
    return nc


def host_prep(x, ptrs_list, seg_list):
    """Host-side sharding + pruning + index preprocessing -> per-core maps."""
    x = np.asarray(x, dtype=np.float32)
    for l, (n_out, f) in enumerate(zip(OUT_SIZES, FANINS)):
        seg = np.asarray(seg_list[l]).astype(np.int64)
        expected = np.repeat(np.arange(n_out, dtype=np.int64), f)
        assert np.array_equal(seg, expected), f"layer {l}: non-uniform segments"

    stages = plan(ptrs_list)
    idx_maps = {f"idx{l}": reorder_wrap(s) for l, s in enumerate(stages)}

    batch = x.shape[1]
    bpc = batch // NCORES
    in_maps = []
    for i in range(NCORES):
        xs = x[:, i * bpc : (i + 1) * bpc]
        # partition p, slot s holds var s*128+p (slot-major var layout)
        xv = np.ascontiguousarray(
            xs.reshape(S_ENC, P, bpc).transpose(1, 0, 2)
        ).reshape(P, -1)
        in_maps.append({"x": xv, **idx_maps})
    return in_maps


def _meta(stages):
    return tuple(
        (s["n_groups"], s["n_src_rows"], tuple(s["chunks"])) for s in stages
    )


_CACHE = {}


def _get_nc(meta=None):
    if meta is None:
        meta = _CACHE.get("meta")
        assert meta is not None, "call kernel() first"
    if _CACHE.get("meta") != meta:
        _CACHE["nc"] = build_nc(meta)
        _CACHE["meta"] = meta
    return _CACHE["nc"]


def kernel(x, ptrs0, seg0, ptrs1, seg1, ptrs2, seg2, ptrs3, seg3):
    from concourse.bass_utils import run_bass_kernel_spmd

    ptrs_list = [ptrs0, ptrs1, ptrs2, ptrs3]
    stages = plan(ptrs_list)
    nc = _get_nc(_meta(stages))
    in_maps = host_prep(x, ptrs_list, [seg0, seg1, seg2, seg3])
    res = run_bass_kernel_spmd(nc, in_maps, core_ids=list(range(NCORES)))
    outs = [r["out"] for r in res.results]
    full = np.concatenate(outs, axis=1)
    # rows were produced in readiness order; map back to natural order
    return np.ascontiguousarray(full[stages[1]["out_perm"]])


# revision 14
# speedup vs baseline: 1.9629x; 1.0005x over previous
"""Trainium2 Bass kernel for the sum-product "knowledge layer" network.

Computation (see problem reference):
  h0 = encode(x): 8194-row table [-inf, 0, pos0, neg0, pos1, neg1, ...]
       with pos = x (log-probs), neg = log(1 - exp(x)), per batch column.
  4 alternating layers, each: gather rows by ptrs, then segment-reduce over
  contiguous fanin groups (fanin 4 sum-of-logs "product" layers, fanin 2
  logsumexp "sum" layers).

Strategy (pure batch data-parallelism, 8 NeuronCores):
  - Shard the 512 batch columns 8 ways -> 64 columns per core.
  - Per core every tensor lives in DRAM as [rows, 64] fp32; one row = 256B.
  - Gathers use the SWDGE dma_gather instruction: int16 index list in SBUF,
    each index pulls one 256B row from the DRAM table; index list position j
    lands at SBUF partition j%128, free slot j//128.
  - DAG pruning (host side, per input set): working back from the 2048
    output rows, only rows actually referenced downstream are computed.
  - Layer fusion: layers 0+1 fuse into stage A, layers 2+3 into stage B.
    A stage group gathers its 8 leaf rows (2 product groups x fanin 4),
    sums each quad on DVE, then logsumexps the pair - so the intermediate
    product table never exists in DRAM, removing its store and a pipeline
    boundary, at the cost of recomputing product rows referenced by more
    than one sum edge (~4% more gather descriptors).
  - Cross-layer software pipelining: stage A's output rows are produced in
    chunk order (chunk ci stores rows [base, base + P*csub), row = base +
    p*csub + cc), A groups are sorted by the max table-0 row they
    reference, and every gather chunk's source AP is narrowed to the exact
    table prefix it needs, so the tile framework only serializes a gather
    against the stores that overlap its prefix. The encode is likewise
    chunked (vars are laid out slot-major: var v lives at partition v%128,
    slot v//128, so encode chunk j fills a table-0 row prefix). Chunk
    sizes ramp up then down so desc-gen and compute tails stay short.
  - Sum reduction: logsumexp(a,b) = a + ln(1+exp(b-a)) in stage A (|b-a|
    < 40, f32-exp-safe) and max + ln(1+exp(min-max)) in stage B (wider
    range), with the Exp+Ln activation table preloaded once (set 6) so
    the compiler inserts no per-instruction table reloads.
"""

import numpy as np

P = 128
B = 64  # batch columns per core
NCORES = 8
N_VARS = 4096
BATCH = 512
TAB0 = 2 * N_VARS + 2  # 8194
OUT_SIZES = [16384, 8192, 4096, 2048]
FANINS = [4, 2, 4, 2]
FE = 8  # edges per fused group: 2 (sum fanin) x 4 (product fanin)
CHUNK = 8192  # max gather indices per dma_gather instruction
S_ENC = N_VARS // P  # 32 encode slots per partition
ENC_CHUNKS = 4
SE = S_ENC // ENC_CHUNKS  # slots per encode chunk


def _pad_groups(n):
    return -(-n // P) * P


def _chunk_group_counts(n_groups):
    """Groups per dma_gather chunk (FE edges per group). Sizes ramp up at
    the start (small first chunk -> quick desc-gen once the source prefix
    lands) and down at the end (short compute tail -> the last store lands
    early and the consumer unblocks sooner)."""
    g = P
    rem = n_groups
    tail = []
    for s in (128, 128, 256, 512):
        if rem >= s + g:
            tail.append(s)
            rem -= s
    head = []
    for s in (128, 256, 512):
        if rem >= s + g:
            head.append(s)
            rem -= s
    mid = []
    per = CHUNK // FE
    while rem > 0:
        s = min(per, rem)
        mid.append(s)
        rem -= s
    return head + mid + tail[::-1]


def plan(ptrs_list):
    """Prune the DAG bottom-up, fuse layer pairs, readiness-order stage A.

    Returns (stageA, stageB) dicts:
      n_groups: padded group count (stage A: pruned sum-layer-1 groups =
                rows of the intermediate table tA; stage B: 2048 outputs)
      n_src_rows: rows of the gathered table (A: TAB0, B: nA)
      chunks: list of (n_groups_in_chunk, src_prefix_rows)
      edge_src: per-edge source rows, production order, FE per group
    """
    p0, p1, p2, p3 = [np.asarray(p).astype(np.int64) for p in ptrs_list]
    # stage B: out group g needs L2 groups p3[2g], p3[2g+1]; each L2 group
    # h needs t-A rows p2[4h+k].
    b_l2 = p3.reshape(-1, 2)  # [2048, 2] L2 group ids
    b_src1 = p2.reshape(-1, 4)[b_l2]  # [2048, 2, 4] L1 (tA) compact... raw ids
    used1 = np.unique(b_src1)
    # stage A: one group per used L1 row; L1 row u needs L0 groups
    # p1[2u+j]; L0 group w needs t0 rows p0[4w+k].
    a_l0 = p1.reshape(-1, 2)[used1]  # [n1, 2] L0 group ids
    a_src0 = p0.reshape(-1, 4)[a_l0]  # [n1, 2, 4] t0 rows

    n1 = used1.size
    nA = _pad_groups(n1)
    srcA = np.zeros((nA, FE), dtype=np.int64)
    srcA[:n1] = a_src0.reshape(n1, FE)
    # readiness of a t0 row: encode chunk order (var slot v//128; rows 0/1
    # ready first)
    ready = np.maximum(srcA - 2, 0) // 2 // P
    order = np.argsort(ready.max(axis=1), kind="stable")
    srcA = srcA[order]
    prod = np.empty(nA, dtype=np.int64)
    prod[order] = np.arange(nA)  # compact A-group id -> production row

    relabel1 = prod[np.searchsorted(used1, b_src1)]  # [2048, 2, 4] tA rows
    srcB = relabel1.reshape(-1, FE)
    # Stage B output rows need no fixed order either - the host unpermutes
    # rows after the run - so readiness-sort B too: its early chunks then
    # only need a tA prefix and can overlap stage A's tail.
    orderB = np.argsort(srcB.max(axis=1), kind="stable")
    srcB = srcB[orderB]
    prodB = np.empty(srcB.shape[0], dtype=np.int64)
    prodB[orderB] = np.arange(srcB.shape[0])  # out group g -> produced row

    def mk(src, n_src_rows):
        n_groups = src.shape[0]
        chunks = []
        g_off = 0
        for gc in _chunk_group_counts(n_groups):
            m = int(src[g_off : g_off + gc].max()) + 1
            chunks.append((gc, m))
            g_off += gc
        return {
            "n_groups": n_groups,
            "n_src_rows": n_src_rows,
            "chunks": chunks,
            "edge_src": src.ravel(),
        }

    stages = [mk(srcA, TAB0), mk(srcB, nA)]
    stages[1]["out_perm"] = prodB
    return stages


def reorder_wrap(stage):
    """Permute per-edge source ids into dma_gather order and wrap into the
    int16 [128, n_edges//16] SBUF layout (position j -> [j%16, j//16],
    replicated across the 8 gpsimd cores' 16-partition groups).

    Edge position j of chunk ci maps to partition p=j%128, slot=j//128,
    cc=slot//FE, k=slot%FE, production row = base_ci + p*csub + cc."""
    src = stage["edge_src"]
    out = np.empty(stage["n_groups"] * FE, dtype=np.int64)
    base = 0
    e_off = 0
    for gc, _m in stage["chunks"]:
        csub = gc // P
        n_e = gc * FE
        j = np.arange(n_e)
        p = j % P
        slot = j // P
        cc = slot // FE
        k = slot % FE
        row = base + p * csub + cc
        out[e_off : e_off + n_e] = src[row * FE + k]
        base += gc
        e_off += n_e
    assert out.max() < 2**15 and out.min() >= 0
    out = out.astype(np.int16)
    return np.ascontiguousarray(np.tile(out.reshape(-1, 16).T, (8, 1)))


def build_nc(meta):
    """meta: per-stage (n_groups, n_src_rows, chunks-tuple)."""
    import concourse.bacc as bacc
    import concourse.mybir as mybir
    import concourse.tile as tile

    f32 = mybir.dt.float32
    i16 = mybir.dt.int16
    Alu = mybir.AluOpType
    Act = mybir.ActivationFunctionType

    specs = [
        {"n_groups": n, "n_src_rows": s, "chunks": ch} for (n, s, ch) in meta
    ]

    nc = bacc.Bacc("TRN2", target_bir_lowering=False, debug=False)
    x = nc.dram_tensor("x", [P, S_ENC * B], f32, kind="ExternalInput")
    idx_in = [
        nc.dram_tensor(
            f"idx{l}", [P, FE * s["n_groups"] // 16], i16, kind="ExternalInput"
        )
        for l, s in enumerate(specs)
    ]
    out = nc.dram_tensor("out", [OUT_SIZES[3], B], f32, kind="ExternalOutput")

    with tile.TileContext(nc) as tc:
        with (
            tc.tile_pool(name="dram", bufs=1, space="DRAM") as dpool,
            tc.tile_pool(name="sb", bufs=5) as gp,
            tc.tile_pool(name="enc", bufs=ENC_CHUNKS) as ep,
            tc.tile_pool(name="hb", bufs=3) as hp,
            tc.tile_pool(name="tmp", bufs=2) as tp,
            tc.tile_pool(name="ix", bufs=1) as ixp,
        ):
            tables = [
                dpool.tile([s["n_src_rows"], B], f32, name=f"t{l}", tag=f"t{l}")
                for l, s in enumerate(specs)
            ]

            # Preload the combined Exp+Ln activation table once; the
            # insert_act_table_loads pass then finds every Exp/Ln already
            # covered and inserts no per-instruction reloads (1283ns each).
            ACT_SET_LN_EXP = 6  # natural_log_exp_and_others
            nc.scalar.add_instruction(
                mybir.InstLoadActFuncSet(
                    name=nc.get_next_instruction_name(),
                    ins=[],
                    outs=[],
                    act_func_set_id=ACT_SET_LN_EXP,
                )
            )

            # table0 rows 0 (-inf in the reference, never gathered) and 1
            # (zeros). Store first so the row prefix [0,2) is ready.
            z = ixp.tile([2, B], f32, tag="z")
            nc.vector.memset(z[:], 0.0)
            nc.sync.dma_start(tables[0][:][0:2, :], z[:])

            # --- encode, chunked: var v sits at partition v%128, slot
            # v//128; pos row 2+2v, neg row 3+2v.  Chunk j covers slots
            # [j*SE,(j+1)*SE) = rows [2+256*j*SE, 2+256*(j+1)*SE), a row
            # prefix, so stage-A gather chunks can start before the whole
            # encode finishes.
            # All independent loads are emitted before any store so the
            # in-order DMA queue never has a compute-gated store blocking a
            # ready load: x chunks first, then the index lists.
            xv = x[:].rearrange("p (s b) -> p s b", b=B)
            xls = []
            for j in range(ENC_CHUNKS):
                # contiguous destination: 2KB DMA descriptors (the
                # interleaved iv[:, :, 0, :] view would halve them to 256B
                # and pay the sub-512B 2x transfer penalty)
                xl = ep.tile([P, SE, B], f32, tag="xl")
                nc.sync.dma_start(xl[:], xv[:, j * SE : (j + 1) * SE, :])
                xls.append(xl)
            ix_t = []
            for l, s in enumerate(specs):
                t = ixp.tile([P, FE * s["n_groups"] // 16], i16, tag=f"ix{l}")
                nc.sync.dma_start(t[:], idx_in[l][:])
                ix_t.append(t)
            for j in range(ENC_CHUNKS):
                xl = xls[j]
                iv = ep.tile([P, SE, 2, B], f32, tag="enc")
                et = hp.tile([P, SE, B], f32, tag="h")
                nc.scalar.copy(iv[:][:, :, 0, :], xl[:])
                nc.scalar.activation(et[:], xl[:], Act.Exp)
                nc.scalar.activation(
                    iv[:][:, :, 1, :], et[:], Act.Ln, scale=-1.0, bias=1.0
                )
                r0 = 2 + 2 * P * SE * j
                r1 = 2 + 2 * P * SE * (j + 1)
                # row = r0 + 256*s + 2*p + k
                nc.sync.dma_start(
                    tables[0][:][r0:r1, :].rearrange("(s p k) b -> p s k b", p=P, k=2),
                    iv[:],
                )

            # --- fused gather + product-sum + logsumexp stages ---
            for l, s in enumerate(specs):
                dst_tile = tables[l + 1][:] if l + 1 < len(specs) else out[:]
                g_off = 0
                e_off = 0
                for gc, m_src in s["chunks"]:
                    csub = gc // P
                    ch = gc * FE
                    S = ch // P
                    g = gp.tile([P, S, B], f32, tag="g")
                    nc.gpsimd.dma_gather(
                        g[:],
                        tables[l][:][0:m_src, :],
                        ix_t[l][:, e_off // 16 : (e_off + ch) // 16],
                        ch,
                        ch,
                        B,
                        single_packet=False,
                    )
                    # [p, group, pair(2), fanin(4), b]
                    v = g[:].rearrange("p (c j k) b -> p c j k b", j=2, k=4)
                    s01 = tp.tile([P, csub, 2, B], f32, tag="m")
                    s23 = tp.tile([P, csub, 2, B], f32, tag="n")
                    ss = gp.tile([P, csub, 2, B], f32, tag="s")
                    nc.vector.tensor_add(s01[:], v[:, :, :, 0, :], v[:, :, :, 1, :])
                    nc.vector.tensor_add(s23[:], v[:, :, :, 2, :], v[:, :, :, 3, :])
                    nc.vector.tensor_add(ss[:], s01[:], s23[:])
                    a = ss[:][:, :, 0, :]
                    b = ss[:][:, :, 1, :]
                    h = hp.tile([P, csub, B], f32, tag="h")
                    if l == 0:
                        # logsumexp(a,b) = a + ln(1+exp(b-a)); |b-a| < 40
                        # here so exp stays in f32 range.
                        d = tp.tile([P, csub, B], f32, tag="d")
                        sp = tp.tile([P, csub, B], f32, tag="sp")
                        nc.vector.tensor_tensor(d[:], b, a, op=Alu.subtract)
                        nc.scalar.activation(d[:], d[:], Act.Exp)
                        nc.scalar.activation(sp[:], d[:], Act.Ln, bias=1.0)
                        nc.vector.tensor_add(h[:], a, sp[:])
                    else:
                        # wider range: logsumexp = max + ln(1+exp(min-max))
                        m = tp.tile([P, csub, B], f32, tag="d")
                        mn = tp.tile([P, csub, B], f32, tag="sp")
                        sp = tp.tile([P, csub, B], f32, tag="sq")
                        nc.vector.tensor_tensor(m[:], a, b, op=Alu.max)
                        nc.vector.tensor_tensor(mn[:], a, b, op=Alu.min)
                        nc.vector.tensor_tensor(mn[:], mn[:], m[:], op=Alu.subtract)
                        nc.scalar.activation(mn[:], mn[:], Act.Exp)
                        nc.scalar.activation(sp[:], mn[:], Act.Ln, bias=1.0)
                        nc.vector.tensor_add(h[:], m[:], sp[:])
                    # chunk produces rows [g_off, g_off + P*csub):
                    # row = g_off + p*csub + cc
                    nc.sync.dma_start(
                        dst_tile[g_off : g_off + P * csub, :].rearrange(
                            "(p c) b -> p (c b)", p=P
                        ),
                        h[:].rearrange("p c b -> p (c b)"),
                    )
                    g_off += P * csub
                    e_off += ch
    nc.compile()
    return nc


def host_prep(x, ptrs_list, seg_list):
    """Host-side sharding + pruning + index preprocessing -> per-core maps."""
    x = np.asarray(x, dtype=np.float32)
    for l, (n_out, f) in enumerate(zip(OUT_SIZES, FANINS)):
        seg = np.asarray(seg_list[l]).astype(np.int64)
        expected = np.repeat(np.arange(n_out, dtype=np.int64), f)
        assert np.array_equal(seg, expected), f"layer {l}: non-uniform segments"

    stages = plan(ptrs_list)
    idx_maps = {f"idx{l}": reorder_wrap(s) for l, s in enumerate(stages)}

    batch = x.shape[1]
    bpc = batch // NCORES
    in_maps = []
    for i in range(NCORES):
        xs = x[:, i * bpc : (i + 1) * bpc]
        # partition p, slot s holds var s*128+p (slot-major var layout)
        xv = np.ascontiguousarray(
            xs.reshape(S_ENC, P, bpc).transpose(1, 0, 2)
        ).reshape(P, -1)
        in_maps.append({"x": xv, **idx_maps})
    return in_maps


def _meta(stages):
    return tuple(
        (s["n_groups"], s["n_src_rows"], tuple(s["chunks"])) for s in stages
    )


_CACHE = {}


def _get_nc(meta=None):
    if meta is None:
        meta = _CACHE.get("meta")
        assert meta is not None, "call kernel() first"
    if _CACHE.get("meta") != meta:
        _CACHE["nc"] = build_nc(meta)
        _CACHE["meta"] = meta
    return _CACHE["nc"]


def kernel(x, ptrs0, seg0, ptrs1, seg1, ptrs2, seg2, ptrs3, seg3):
    from concourse.bass_utils import run_bass_kernel_spmd

    ptrs_list = [ptrs0, ptrs1, ptrs2, ptrs3]
    stages = plan(ptrs_list)
    nc = _get_nc(_meta(stages))
    in_maps = host_prep(x, ptrs_list, [seg0, seg1, seg2, seg3])
    res = run_bass_kernel_spmd(nc, in_maps, core_ids=list(range(NCORES)))
    outs = [r["out"] for r in res.results]
    full = np.concatenate(outs, axis=1)
    # rows were produced in readiness order; map back to natural order
    return np.ascontiguousarray(full[stages[1]["out_perm"]])


# revision 21
# speedup vs baseline: 1.9774x; 1.0074x over previous
"""Trainium2 Bass kernel for the sum-product "knowledge layer" network.

Computation (see problem reference):
  h0 = encode(x): 8194-row table [-inf, 0, pos0, neg0, pos1, neg1, ...]
       with pos = x (log-probs), neg = log(1 - exp(x)), per batch column.
  4 alternating layers, each: gather rows by ptrs, then segment-reduce over
  contiguous fanin groups (fanin 4 sum-of-logs "product" layers, fanin 2
  logsumexp "sum" layers).

Strategy (pure batch data-parallelism, 8 NeuronCores):
  - Shard the 512 batch columns 8 ways -> 64 columns per core.
  - Per core every tensor lives in DRAM as [rows, 64] fp32; one row = 256B.
  - Gathers use the SWDGE dma_gather instruction: int16 index list in SBUF,
    each index pulls one 256B row from the DRAM table; index list position j
    lands at SBUF partition j%128, free slot j//128.
  - DAG pruning (host side, per input set): working back from the 2048
    output rows, only rows actually referenced downstream are computed.
  - Layer fusion: layers 0+1 fuse into stage A, layers 2+3 into stage B.
    A stage group gathers its 8 leaf rows (2 product groups x fanin 4),
    sums each quad on DVE, then logsumexps the pair - so the intermediate
    product table never exists in DRAM, removing its store and a pipeline
    boundary, at the cost of recomputing product rows referenced by more
    than one sum edge (~4% more gather descriptors).
  - Cross-layer software pipelining: stage A's output rows are produced in
    chunk order (chunk ci stores rows [base, base + P*csub), row = base +
    p*csub + cc), A groups are sorted by the max table-0 row they
    reference, and every gather chunk's source AP is narrowed to the exact
    table prefix it needs, so the tile framework only serializes a gather
    against the stores that overlap its prefix. The encode is likewise
    chunked (vars are laid out slot-major: var v lives at partition v%128,
    slot v//128, so encode chunk j fills a table-0 row prefix). Chunk
    sizes ramp up then down so desc-gen and compute tails stay short.
  - Sum reduction: logsumexp(a,b) = a + ln(1+exp(b-a)) in stage A (|b-a|
    < 40, f32-exp-safe) and max + ln(1+exp(min-max)) in stage B (wider
    range), with the Exp+Ln activation table preloaded once (set 6) so
    the compiler inserts no per-instruction table reloads.
"""

import numpy as np

P = 128
B = 64  # batch columns per core
NCORES = 8
N_VARS = 4096
BATCH = 512
TAB0 = 2 * N_VARS + 2  # 8194
OUT_SIZES = [16384, 8192, 4096, 2048]
FANINS = [4, 2, 4, 2]
FE = 8  # edges per fused group: 2 (sum fanin) x 4 (product fanin)
CHUNK = 8192  # max gather indices per dma_gather instruction
S_ENC = N_VARS // P  # 32 encode slots per partition
ENC_CHUNKS = 4
SE = S_ENC // ENC_CHUNKS  # slots per encode chunk


def _pad_groups(n):
    return -(-n // P) * P


def _chunk_group_counts(n_groups):
    """Groups per dma_gather chunk (FE edges per group). Sizes ramp up at
    the start (small first chunk -> quick desc-gen once the source prefix
    lands) and down at the end (short compute tail -> the last store lands
    early and the consumer unblocks sooner)."""
    g = P
    rem = n_groups
    tail = []
    for s in (128, 128, 256, 512):
        if rem >= s + g:
            tail.append(s)
            rem -= s
    head = []
    for s in (128, 256, 512):
        if rem >= s + g:
            head.append(s)
            rem -= s
    mid = []
    per = CHUNK // FE
    while rem > 0:
        s = min(per, rem)
        mid.append(s)
        rem -= s
    return head + mid + tail[::-1]


def plan(ptrs_list):
    """Prune the DAG bottom-up, fuse layer pairs, readiness-order stage A.

    Returns (stageA, stageB) dicts:
      n_groups: padded group count (stage A: pruned sum-layer-1 groups =
                rows of the intermediate table tA; stage B: 2048 outputs)
      n_src_rows: rows of the gathered table (A: TAB0, B: nA)
      chunks: list of (n_groups_in_chunk, src_prefix_rows)
      edge_src: per-edge source rows, production order, FE per group
    """
    p0, p1, p2, p3 = [np.asarray(p).astype(np.int64) for p in ptrs_list]
    # stage B: out group g needs L2 groups p3[2g], p3[2g+1]; each L2 group
    # h needs t-A rows p2[4h+k].
    b_l2 = p3.reshape(-1, 2)  # [2048, 2] L2 group ids
    b_src1 = p2.reshape(-1, 4)[b_l2]  # [2048, 2, 4] L1 (tA) compact... raw ids
    used1 = np.unique(b_src1)
    # stage A: one group per used L1 row; L1 row u needs L0 groups
    # p1[2u+j]; L0 group w needs t0 rows p0[4w+k].
    a_l0 = p1.reshape(-1, 2)[used1]  # [n1, 2] L0 group ids
    a_src0 = p0.reshape(-1, 4)[a_l0]  # [n1, 2, 4] t0 rows

    n1 = used1.size
    nA = _pad_groups(n1)
    srcA = np.zeros((nA, FE), dtype=np.int64)
    srcA[:n1] = a_src0.reshape(n1, FE)
    # readiness of a t0 row: encode chunk order (var slot v//128; rows 0/1
    # ready first)
    ready = np.maximum(srcA - 2, 0) // 2 // P
    rmax = ready.max(axis=1)
    # padding groups read only row 0: ready before any encode chunk, so
    # putting them first lets chunk 0's desc-gen+gather warm up under the
    # encode instead of idling the DMA engines.
    rmax[n1:] = -1
    order = np.argsort(rmax, kind="stable")
    srcA = srcA[order]
    prod = np.empty(nA, dtype=np.int64)
    prod[order] = np.arange(nA)  # compact A-group id -> production row

    relabel1 = prod[np.searchsorted(used1, b_src1)]  # [2048, 2, 4] tA rows
    srcB = relabel1.reshape(-1, FE)
    # Stage B output rows need no fixed order either - the host unpermutes
    # rows after the run - so readiness-sort B too: its early chunks then
    # only need a tA prefix and can overlap stage A's tail.
    orderB = np.argsort(srcB.max(axis=1), kind="stable")
    srcB = srcB[orderB]
    prodB = np.empty(srcB.shape[0], dtype=np.int64)
    prodB[orderB] = np.arange(srcB.shape[0])  # out group g -> produced row

    def mk(src, n_src_rows):
        n_groups = src.shape[0]
        chunks = []
        g_off = 0
        for gc in _chunk_group_counts(n_groups):
            m = int(src[g_off : g_off + gc].max()) + 1
            chunks.append((gc, m))
            g_off += gc
        return {
            "n_groups": n_groups,
            "n_src_rows": n_src_rows,
            "chunks": chunks,
            "edge_src": src.ravel(),
        }

    stages = [mk(srcA, TAB0), mk(srcB, nA)]
    stages[1]["out_perm"] = prodB

    # Logsumexp form per stage: lse(a,b) = ln(e^a + e^b) directly costs
    # 1 DVE op (vs 4) + whole-tile Exp on the lightly-loaded ACT engine,
    # but is only safe when e^min stays a normal f32.  resolve_direct()
    # checks the exact values once x is known.
    stages[0]["direct"] = None
    stages[1]["direct"] = None
    stages[0]["_srcB_quads"] = srcB.reshape(-1, 2, 4)
    return stages


def resolve_direct(stages, x):
    """Exact host-side bound check for the stage-B direct-form lse."""
    x = np.asarray(x, dtype=np.float64)
    pos = x
    neg = np.log1p(-np.exp(x))
    t0 = np.empty((TAB0, x.shape[1]))
    t0[0] = 0.0
    t0[1] = 0.0
    t0[2::2] = pos
    t0[3::2] = neg
    srcA = stages[0]["edge_src"].reshape(-1, 2, 4)
    q = t0[srcA].sum(axis=2)  # [nA, 2, cols]
    m = q.max(axis=1)
    mn = q.min(axis=1)
    tA = m + np.log1p(np.exp(mn - m))
    tb = tA[stages[0]["_srcB_quads"]].sum(axis=2)  # [2048, 2, cols]
    safe_a = q.min() > -80.0
    safe_b = tb.min() > -80.0
    stages[0]["direct"] = bool(safe_a)
    stages[1]["direct"] = bool(safe_b)


def reorder_wrap(stage):
    """Permute per-edge source ids into dma_gather order and wrap into the
    int16 [128, n_edges//16] SBUF layout (position j -> [j%16, j//16],
    replicated across the 8 gpsimd cores' 16-partition groups).

    Edge position j of chunk ci maps to partition p=j%128, slot=j//128,
    cc=slot//FE, k=slot%FE, production row = base_ci + p*csub + cc."""
    src = stage["edge_src"]
    out = np.empty(stage["n_groups"] * FE, dtype=np.int64)
    base = 0
    e_off = 0
    for gc, _m in stage["chunks"]:
        csub = gc // P
        n_e = gc * FE
        j = np.arange(n_e)
        p = j % P
        slot = j // P
        cc = slot // FE
        k = slot % FE
        row = base + p * csub + cc
        out[e_off : e_off + n_e] = src[row * FE + k]
        base += gc
        e_off += n_e
    assert out.max() < 2**15 and out.min() >= 0
    out = out.astype(np.int16)
    return np.ascontiguousarray(np.tile(out.reshape(-1, 16).T, (8, 1)))


def build_nc(meta):
    """meta: per-stage (n_groups, n_src_rows, chunks-tuple)."""
    import concourse.bacc as bacc
    import concourse.mybir as mybir
    import concourse.tile as tile

    f32 = mybir.dt.float32
    i16 = mybir.dt.int16
    Alu = mybir.AluOpType
    Act = mybir.ActivationFunctionType

    specs = [
        {"n_groups": n, "n_src_rows": s, "chunks": ch, "direct": dr}
        for (n, s, ch, dr) in meta
    ]

    nc = bacc.Bacc("TRN2", target_bir_lowering=False, debug=False)
    x = nc.dram_tensor("x", [P, S_ENC * B], f32, kind="ExternalInput")
    idx_in = [
        nc.dram_tensor(
            f"idx{l}", [P, FE * s["n_groups"] // 16], i16, kind="ExternalInput"
        )
        for l, s in enumerate(specs)
    ]
    out = nc.dram_tensor("out", [OUT_SIZES[3], B], f32, kind="ExternalOutput")

    with tile.TileContext(nc) as tc:
        with (
            tc.tile_pool(name="dram", bufs=1, space="DRAM") as dpool,
            tc.tile_pool(name="sb", bufs=5) as gp,
            tc.tile_pool(name="enc", bufs=ENC_CHUNKS) as ep,
            tc.tile_pool(name="hb", bufs=3) as hp,
            tc.tile_pool(name="tmp", bufs=2) as tp,
            tc.tile_pool(name="ix", bufs=1) as ixp,
        ):
            tables = [
                dpool.tile([s["n_src_rows"], B], f32, name=f"t{l}", tag=f"t{l}")
                for l, s in enumerate(specs)
            ]

            # Preload the combined Exp+Ln activation table once; the
            # insert_act_table_loads pass then finds every Exp/Ln already
            # covered and inserts no per-instruction reloads (1283ns each).
            ACT_SET_LN_EXP = 6  # natural_log_exp_and_others
            nc.scalar.add_instruction(
                mybir.InstLoadActFuncSet(
                    name=nc.get_next_instruction_name(),
                    ins=[],
                    outs=[],
                    act_func_set_id=ACT_SET_LN_EXP,
                )
            )

            # table0 rows 0 (-inf in the reference, never gathered) and 1
            # (zeros). Store first so the row prefix [0,2) is ready.
            z = ixp.tile([2, B], f32, tag="z")
            nc.vector.memset(z[:], 0.0)
            nc.sync.dma_start(tables[0][:][0:2, :], z[:])

            # --- encode, chunked: var v sits at partition v%128, slot
            # v//128; pos row 2+2v, neg row 3+2v.  Chunk j covers slots
            # [j*SE,(j+1)*SE) = rows [2+256*j*SE, 2+256*(j+1)*SE), a row
            # prefix, so stage-A gather chunks can start before the whole
            # encode finishes.
            # All independent loads are emitted before any store so the
            # in-order DMA queue never has a compute-gated store blocking a
            # ready load: x chunks first, then the index lists.
            xv = x[:].rearrange("p (s b) -> p s b", b=B)
            xls = []
            for j in range(ENC_CHUNKS):
                # contiguous destination: 2KB DMA descriptors (the
                # interleaved iv[:, :, 0, :] view would halve them to 256B
                # and pay the sub-512B 2x transfer penalty)
                xl = ep.tile([P, SE, B], f32, tag="xl")
                nc.sync.dma_start(xl[:], xv[:, j * SE : (j + 1) * SE, :])
                xls.append(xl)
            ix_t = []
            for l, s in enumerate(specs):
                t = ixp.tile([P, FE * s["n_groups"] // 16], i16, tag=f"ix{l}")
                nc.sync.dma_start(t[:], idx_in[l][:])
                ix_t.append(t)
            for j in range(ENC_CHUNKS):
                xl = xls[j]
                iv = ep.tile([P, SE, 2, B], f32, tag="enc")
                et = hp.tile([P, SE, B], f32, tag="h")
                nc.scalar.copy(iv[:][:, :, 0, :], xl[:])
                nc.scalar.activation(et[:], xl[:], Act.Exp)
                nc.scalar.activation(
                    iv[:][:, :, 1, :], et[:], Act.Ln, scale=-1.0, bias=1.0
                )
                r0 = 2 + 2 * P * SE * j
                r1 = 2 + 2 * P * SE * (j + 1)
                # row = r0 + 256*s + 2*p + k
                nc.sync.dma_start(
                    tables[0][:][r0:r1, :].rearrange("(s p k) b -> p s k b", p=P, k=2),
                    iv[:],
                )

            # --- fused gather + product-sum + logsumexp stages ---
            for l, s in enumerate(specs):
                dst_tile = tables[l + 1][:] if l + 1 < len(specs) else out[:]
                g_off = 0
                e_off = 0
                for gc, m_src in s["chunks"]:
                    csub = gc // P
                    ch = gc * FE
                    S = ch // P
                    g = gp.tile([P, S, B], f32, tag="g")
                    nc.gpsimd.dma_gather(
                        g[:],
                        tables[l][:][0:m_src, :],
                        ix_t[l][:, e_off // 16 : (e_off + ch) // 16],
                        ch,
                        ch,
                        B,
                        single_packet=False,
                    )
                    # [p, group, pair(2), fanin(4), b]
                    v = g[:].rearrange("p (c j k) b -> p c j k b", j=2, k=4)
                    s01 = tp.tile([P, csub, 2, B], f32, tag="m")
                    s23 = tp.tile([P, csub, 2, B], f32, tag="n")
                    ss = gp.tile([P, csub, 2, B], f32, tag="s")
                    nc.vector.tensor_add(s01[:], v[:, :, :, 0, :], v[:, :, :, 1, :])
                    nc.vector.tensor_add(s23[:], v[:, :, :, 2, :], v[:, :, :, 3, :])
                    nc.vector.tensor_add(ss[:], s01[:], s23[:])
                    a = ss[:][:, :, 0, :]
                    b = ss[:][:, :, 1, :]
                    h = hp.tile([P, csub, B], f32, tag="h")
                    if s["direct"]:
                        # lse(a,b) = ln(e^a + e^b): host verified e^min is a
                        # normal f32 (no scaling needed). 1 DVE op; the
                        # whole-tile Exp and the Ln ride the ACT engine.
                        e = tp.tile([P, csub, 2, B], f32, tag="d")
                        d = tp.tile([P, csub, B], f32, tag="sp")
                        nc.scalar.activation(e[:], ss[:], Act.Exp)
                        nc.vector.tensor_add(
                            d[:], e[:][:, :, 0, :], e[:][:, :, 1, :]
                        )
                        nc.scalar.activation(h[:], d[:], Act.Ln)
                    else:
                        # wider range: logsumexp = max + ln(1+exp(min-max))
                        m = tp.tile([P, csub, B], f32, tag="d")
                        mn = tp.tile([P, csub, B], f32, tag="sp")
                        sp = tp.tile([P, csub, B], f32, tag="sq")
                        nc.vector.tensor_tensor(m[:], a, b, op=Alu.max)
                        nc.vector.tensor_tensor(mn[:], a, b, op=Alu.min)
                        nc.vector.tensor_tensor(mn[:], mn[:], m[:], op=Alu.subtract)
                        nc.scalar.activation(mn[:], mn[:], Act.Exp)
                        nc.scalar.activation(sp[:], mn[:], Act.Ln, bias=1.0)
                        nc.vector.tensor_add(h[:], m[:], sp[:])
                    # chunk produces rows [g_off, g_off + P*csub):
                    # row = g_off + p*csub + cc
                    nc.sync.dma_start(
                        dst_tile[g_off : g_off + P * csub, :].rearrange(
                            "(p c) b -> p (c b)", p=P
                        ),
                        h[:].rearrange("p c b -> p (c b)"),
                    )
                    g_off += P * csub
                    e_off += ch
    nc.compile()
    return nc


def host_prep(x, ptrs_list, seg_list):
    """Host-side sharding + pruning + index preprocessing -> per-core maps."""
    x = np.asarray(x, dtype=np.float32)
    for l, (n_out, f) in enumerate(zip(OUT_SIZES, FANINS)):
        seg = np.asarray(seg_list[l]).astype(np.int64)
        expected = np.repeat(np.arange(n_out, dtype=np.int64), f)
        assert np.array_equal(seg, expected), f"layer {l}: non-uniform segments"

    stages = plan(ptrs_list)
    idx_maps = {f"idx{l}": reorder_wrap(s) for l, s in enumerate(stages)}

    batch = x.shape[1]
    bpc = batch // NCORES
    in_maps = []
    for i in range(NCORES):
        xs = x[:, i * bpc : (i + 1) * bpc]
        # partition p, slot s holds var s*128+p (slot-major var layout)
        xv = np.ascontiguousarray(
            xs.reshape(S_ENC, P, bpc).transpose(1, 0, 2)
        ).reshape(P, -1)
        in_maps.append({"x": xv, **idx_maps})
    return in_maps


def _meta(stages):
    return tuple(
        (s["n_groups"], s["n_src_rows"], tuple(s["chunks"]), bool(s["direct"]))
        for s in stages
    )


_CACHE = {}


def _get_nc(meta=None):
    if meta is None:
        meta = _CACHE.get("meta")
        assert meta is not None, "call kernel() first"
    if _CACHE.get("meta") != meta:
        _CACHE["nc"] = build_nc(meta)
        _CACHE["meta"] = meta
    return _CACHE["nc"]


def kernel(x, ptrs0, seg0, ptrs1, seg1, ptrs2, seg2, ptrs3, seg3):
    from concourse.bass_utils import run_bass_kernel_spmd

    ptrs_list = [ptrs0, ptrs1, ptrs2, ptrs3]
    stages = plan(ptrs_list)
    resolve_direct(stages, x)
    nc = _get_nc(_meta(stages))
    in_maps = host_prep(x, ptrs_list, [seg0, seg1, seg2, seg3])
    res = run_bass_kernel_spmd(nc, in_maps, core_ids=list(range(NCORES)))
    outs = [r["out"] for r in res.results]
    full = np.concatenate(outs, axis=1)
    # rows were produced in readiness order; map back to natural order
    return np.ascontiguousarray(full[stages[1]["out_perm"]])


# revision 23
# speedup vs baseline: 1.9929x; 1.0079x over previous
"""Trainium2 Bass kernel for the sum-product "knowledge layer" network.

Computation (see problem reference):
  h0 = encode(x): 8194-row table [-inf, 0, pos0, neg0, pos1, neg1, ...]
       with pos = x (log-probs), neg = log(1 - exp(x)), per batch column.
  4 alternating layers, each: gather rows by ptrs, then segment-reduce over
  contiguous fanin groups (fanin 4 sum-of-logs "product" layers, fanin 2
  logsumexp "sum" layers).

Strategy (pure batch data-parallelism, 8 NeuronCores):
  - Shard the 512 batch columns 8 ways -> 64 columns per core.
  - Per core every tensor lives in DRAM as [rows, 64] fp32; one row = 256B.
  - Gathers use the SWDGE dma_gather instruction: int16 index list in SBUF,
    each index pulls one 256B row from the DRAM table; index list position j
    lands at SBUF partition j%128, free slot j//128.
  - DAG pruning (host side, per input set): working back from the 2048
    output rows, only rows actually referenced downstream are computed.
  - Layer fusion: layers 0+1 fuse into stage A, layers 2+3 into stage B.
    A stage group gathers its 8 leaf rows (2 product groups x fanin 4),
    sums each quad on DVE, then logsumexps the pair - so the intermediate
    product table never exists in DRAM, removing its store and a pipeline
    boundary, at the cost of recomputing product rows referenced by more
    than one sum edge (~4% more gather descriptors).
  - Cross-layer software pipelining: stage A's output rows are produced in
    chunk order (chunk ci stores rows [base, base + P*csub), row = base +
    p*csub + cc), A groups are sorted by the max table-0 row they
    reference, and every gather chunk's source AP is narrowed to the exact
    table prefix it needs, so the tile framework only serializes a gather
    against the stores that overlap its prefix. The encode is likewise
    chunked (vars are laid out slot-major: var v lives at partition v%128,
    slot v//128, so encode chunk j fills a table-0 row prefix). Chunk
    sizes ramp up then down so desc-gen and compute tails stay short.
  - Sum reduction: lse(a,b) = ln(e^a + e^b) computed directly (1 DVE add;
    Exp/Ln on the lightly-loaded ACT engine) - resolve_direct() verifies
    on the host, from the actual x values, that every e^arg stays a
    normal f32, falling back to max + ln(1+exp(min-max)) otherwise.  The
    Exp+Ln activation table is preloaded once (set 6) so the compiler
    inserts no per-instruction table reloads.
"""

import numpy as np

P = 128
B = 64  # batch columns per core
NCORES = 8
N_VARS = 4096
BATCH = 512
TAB0 = 2 * N_VARS + 2  # 8194
OUT_SIZES = [16384, 8192, 4096, 2048]
FANINS = [4, 2, 4, 2]
FE = 8  # edges per fused group: 2 (sum fanin) x 4 (product fanin)
CHUNK = 8192  # max gather indices per dma_gather instruction
S_ENC = N_VARS // P  # 32 encode slots per partition
ENC_CHUNKS = 4
SE = S_ENC // ENC_CHUNKS  # slots per encode chunk


def _pad_groups(n):
    return -(-n // P) * P


def _chunk_group_counts(n_groups):
    """Groups per dma_gather chunk (FE edges per group). Sizes ramp up at
    the start (small first chunk -> quick desc-gen once the source prefix
    lands) and down at the end (short compute tail -> the last store lands
    early and the consumer unblocks sooner)."""
    g = P
    rem = n_groups
    tail = []
    for s in (128, 128, 128, 256, 512):
        if rem >= s + g:
            tail.append(s)
            rem -= s
    head = []
    for s in (128, 256, 512):
        if rem >= s + g:
            head.append(s)
            rem -= s
    mid = []
    per = CHUNK // FE
    while rem > 0:
        s = min(per, rem)
        mid.append(s)
        rem -= s
    return head + mid + tail[::-1]


def plan(ptrs_list):
    """Prune the DAG bottom-up, fuse layer pairs, readiness-order stage A.

    Returns (stageA, stageB) dicts:
      n_groups: padded group count (stage A: pruned sum-layer-1 groups =
                rows of the intermediate table tA; stage B: 2048 outputs)
      n_src_rows: rows of the gathered table (A: TAB0, B: nA)
      chunks: list of (n_groups_in_chunk, src_prefix_rows)
      edge_src: per-edge source rows, production order, FE per group
    """
    p0, p1, p2, p3 = [np.asarray(p).astype(np.int64) for p in ptrs_list]
    # stage B: out group g needs L2 groups p3[2g], p3[2g+1]; each L2 group
    # h needs t-A rows p2[4h+k].
    b_l2 = p3.reshape(-1, 2)  # [2048, 2] L2 group ids
    b_src1 = p2.reshape(-1, 4)[b_l2]  # [2048, 2, 4] L1 (tA) compact... raw ids
    used1 = np.unique(b_src1)
    # stage A: one group per used L1 row; L1 row u needs L0 groups
    # p1[2u+j]; L0 group w needs t0 rows p0[4w+k].
    a_l0 = p1.reshape(-1, 2)[used1]  # [n1, 2] L0 group ids
    a_src0 = p0.reshape(-1, 4)[a_l0]  # [n1, 2, 4] t0 rows

    n1 = used1.size
    nA = _pad_groups(n1)
    srcA = np.zeros((nA, FE), dtype=np.int64)
    srcA[:n1] = a_src0.reshape(n1, FE)
    # readiness of a t0 row: encode chunk order (var slot v//128; rows 0/1
    # ready first)
    ready = np.maximum(srcA - 2, 0) // 2 // P
    rmax = ready.max(axis=1)
    # padding groups read only row 0: ready before any encode chunk, so
    # putting them first lets chunk 0's desc-gen+gather warm up under the
    # encode instead of idling the DMA engines.
    rmax[n1:] = -1
    order = np.argsort(rmax, kind="stable")
    srcA = srcA[order]
    prod = np.empty(nA, dtype=np.int64)
    prod[order] = np.arange(nA)  # compact A-group id -> production row

    relabel1 = prod[np.searchsorted(used1, b_src1)]  # [2048, 2, 4] tA rows
    srcB = relabel1.reshape(-1, FE)
    # Stage B output rows need no fixed order either - the host unpermutes
    # rows after the run - so readiness-sort B too: its early chunks then
    # only need a tA prefix and can overlap stage A's tail.
    orderB = np.argsort(srcB.max(axis=1), kind="stable")
    srcB = srcB[orderB]
    prodB = np.empty(srcB.shape[0], dtype=np.int64)
    prodB[orderB] = np.arange(srcB.shape[0])  # out group g -> produced row

    def mk(src, n_src_rows):
        n_groups = src.shape[0]
        chunks = []
        g_off = 0
        for gc in _chunk_group_counts(n_groups):
            m = int(src[g_off : g_off + gc].max()) + 1
            chunks.append((gc, m))
            g_off += gc
        return {
            "n_groups": n_groups,
            "n_src_rows": n_src_rows,
            "chunks": chunks,
            "edge_src": src.ravel(),
        }

    stages = [mk(srcA, TAB0), mk(srcB, nA)]
    stages[1]["out_perm"] = prodB

    # Logsumexp form per stage: lse(a,b) = ln(e^a + e^b) directly costs
    # 1 DVE op (vs 4) + whole-tile Exp on the lightly-loaded ACT engine,
    # but is only safe when e^min stays a normal f32.  resolve_direct()
    # checks the exact values once x is known.
    stages[0]["direct"] = None
    stages[1]["direct"] = None
    stages[0]["_srcB_quads"] = srcB.reshape(-1, 2, 4)
    return stages


def resolve_direct(stages, x):
    """Exact host-side bound check for the stage-B direct-form lse."""
    x = np.asarray(x, dtype=np.float64)
    pos = x
    neg = np.log1p(-np.exp(x))
    t0 = np.empty((TAB0, x.shape[1]))
    t0[0] = 0.0
    t0[1] = 0.0
    t0[2::2] = pos
    t0[3::2] = neg
    srcA = stages[0]["edge_src"].reshape(-1, 2, 4)
    q = t0[srcA].sum(axis=2)  # [nA, 2, cols]
    m = q.max(axis=1)
    mn = q.min(axis=1)
    tA = m + np.log1p(np.exp(mn - m))
    tb = tA[stages[0]["_srcB_quads"]].sum(axis=2)  # [2048, 2, cols]
    safe_a = q.min() > -80.0
    safe_b = tb.min() > -80.0
    stages[0]["direct"] = bool(safe_a)
    stages[1]["direct"] = bool(safe_b)


def reorder_wrap(stage):
    """Permute per-edge source ids into dma_gather order and wrap into the
    int16 [128, n_edges//16] SBUF layout (position j -> [j%16, j//16],
    replicated across the 8 gpsimd cores' 16-partition groups).

    Edge position j of chunk ci maps to partition p=j%128, slot=j//128,
    cc=slot//FE, k=slot%FE, production row = base_ci + p*csub + cc."""
    src = stage["edge_src"]
    out = np.empty(stage["n_groups"] * FE, dtype=np.int64)
    base = 0
    e_off = 0
    for gc, _m in stage["chunks"]:
        csub = gc // P
        n_e = gc * FE
        j = np.arange(n_e)
        p = j % P
        slot = j // P
        cc = slot // FE
        k = slot % FE
        row = base + p * csub + cc
        out[e_off : e_off + n_e] = src[row * FE + k]
        base += gc
        e_off += n_e
    assert out.max() < 2**15 and out.min() >= 0
    out = out.astype(np.int16)
    return np.ascontiguousarray(np.tile(out.reshape(-1, 16).T, (8, 1)))


def build_nc(meta):
    """meta: per-stage (n_groups, n_src_rows, chunks-tuple)."""
    import concourse.bacc as bacc
    import concourse.mybir as mybir
    import concourse.tile as tile

    f32 = mybir.dt.float32
    i16 = mybir.dt.int16
    Alu = mybir.AluOpType
    Act = mybir.ActivationFunctionType

    specs = [
        {"n_groups": n, "n_src_rows": s, "chunks": ch, "direct": dr}
        for (n, s, ch, dr) in meta
    ]

    nc = bacc.Bacc("TRN2", target_bir_lowering=False, debug=False)
    x = nc.dram_tensor("x", [P, S_ENC * B], f32, kind="ExternalInput")
    idx_in = [
        nc.dram_tensor(
            f"idx{l}", [P, FE * s["n_groups"] // 16], i16, kind="ExternalInput"
        )
        for l, s in enumerate(specs)
    ]
    out = nc.dram_tensor("out", [OUT_SIZES[3], B], f32, kind="ExternalOutput")

    with tile.TileContext(nc) as tc:
        with (
            tc.tile_pool(name="dram", bufs=1, space="DRAM") as dpool,
            tc.tile_pool(name="sb", bufs=5) as gp,
            tc.tile_pool(name="enc", bufs=ENC_CHUNKS) as ep,
            tc.tile_pool(name="hb", bufs=4) as hp,
            tc.tile_pool(name="tmp", bufs=2) as tp,
            tc.tile_pool(name="ix", bufs=1) as ixp,
        ):
            tables = [
                dpool.tile([s["n_src_rows"], B], f32, name=f"t{l}", tag=f"t{l}")
                for l, s in enumerate(specs)
            ]

            # Preload the combined Exp+Ln activation table once; the
            # insert_act_table_loads pass then finds every Exp/Ln already
            # covered and inserts no per-instruction reloads (1283ns each).
            ACT_SET_LN_EXP = 6  # natural_log_exp_and_others
            nc.scalar.add_instruction(
                mybir.InstLoadActFuncSet(
                    name=nc.get_next_instruction_name(),
                    ins=[],
                    outs=[],
                    act_func_set_id=ACT_SET_LN_EXP,
                )
            )

            # table0 rows 0 (-inf in the reference, never gathered) and 1
            # (zeros). Store first so the row prefix [0,2) is ready.
            z = ixp.tile([2, B], f32, tag="z")
            nc.vector.memset(z[:], 0.0)
            nc.sync.dma_start(tables[0][:][0:2, :], z[:])

            # --- encode, chunked: var v sits at partition v%128, slot
            # v//128; pos row 2+2v, neg row 3+2v.  Chunk j covers slots
            # [j*SE,(j+1)*SE) = rows [2+256*j*SE, 2+256*(j+1)*SE), a row
            # prefix, so stage-A gather chunks can start before the whole
            # encode finishes.
            # All independent loads are emitted before any store so the
            # in-order DMA queue never has a compute-gated store blocking a
            # ready load: x chunks first, then the index lists.
            xv = x[:].rearrange("p (s b) -> p s b", b=B)
            xls = []
            for j in range(ENC_CHUNKS):
                # contiguous destination: 2KB DMA descriptors (the
                # interleaved iv[:, :, 0, :] view would halve them to 256B
                # and pay the sub-512B 2x transfer penalty)
                xl = ep.tile([P, SE, B], f32, tag="xl")
                nc.sync.dma_start(xl[:], xv[:, j * SE : (j + 1) * SE, :])
                xls.append(xl)
            ix_t = []
            for l, s in enumerate(specs):
                t = ixp.tile([P, FE * s["n_groups"] // 16], i16, tag=f"ix{l}")
                nc.sync.dma_start(t[:], idx_in[l][:])
                ix_t.append(t)
            for j in range(ENC_CHUNKS):
                xl = xls[j]
                iv = ep.tile([P, SE, 2, B], f32, tag="enc")
                et = hp.tile([P, SE, B], f32, tag="h")
                nc.scalar.copy(iv[:][:, :, 0, :], xl[:])
                nc.scalar.activation(et[:], xl[:], Act.Exp)
                nc.scalar.activation(
                    iv[:][:, :, 1, :], et[:], Act.Ln, scale=-1.0, bias=1.0
                )
                r0 = 2 + 2 * P * SE * j
                r1 = 2 + 2 * P * SE * (j + 1)
                # row = r0 + 256*s + 2*p + k
                nc.sync.dma_start(
                    tables[0][:][r0:r1, :].rearrange("(s p k) b -> p s k b", p=P, k=2),
                    iv[:],
                )

            # --- fused gather + product-sum + logsumexp stages ---
            for l, s in enumerate(specs):
                dst_tile = tables[l + 1][:] if l + 1 < len(specs) else out[:]
                g_off = 0
                e_off = 0
                for gc, m_src in s["chunks"]:
                    csub = gc // P
                    ch = gc * FE
                    S = ch // P
                    g = gp.tile([P, S, B], f32, tag="g")
                    nc.gpsimd.dma_gather(
                        g[:],
                        tables[l][:][0:m_src, :],
                        ix_t[l][:, e_off // 16 : (e_off + ch) // 16],
                        ch,
                        ch,
                        B,
                        single_packet=False,
                    )
                    # [p, group, pair(2), fanin(4), b]
                    v = g[:].rearrange("p (c j k) b -> p c j k b", j=2, k=4)
                    s01 = tp.tile([P, csub, 2, B], f32, tag="m")
                    s23 = tp.tile([P, csub, 2, B], f32, tag="n")
                    ss = gp.tile([P, csub, 2, B], f32, tag="s")
                    nc.vector.tensor_add(s01[:], v[:, :, :, 0, :], v[:, :, :, 1, :])
                    nc.vector.tensor_add(s23[:], v[:, :, :, 2, :], v[:, :, :, 3, :])
                    nc.vector.tensor_add(ss[:], s01[:], s23[:])
                    a = ss[:][:, :, 0, :]
                    b = ss[:][:, :, 1, :]
                    h = hp.tile([P, csub, B], f32, tag="h")
                    if s["direct"]:
                        # lse(a,b) = ln(e^a + e^b): host verified e^min is a
                        # normal f32 (no scaling needed). 1 DVE op; the
                        # whole-tile Exp and the Ln ride the ACT engine.
                        e = tp.tile([P, csub, 2, B], f32, tag="d")
                        d = tp.tile([P, csub, B], f32, tag="sp")
                        nc.scalar.activation(e[:], ss[:], Act.Exp)
                        nc.vector.tensor_add(
                            d[:], e[:][:, :, 0, :], e[:][:, :, 1, :]
                        )
                        nc.scalar.activation(h[:], d[:], Act.Ln)
                    else:
                        # wider range: logsumexp = max + ln(1+exp(min-max))
                        m = tp.tile([P, csub, B], f32, tag="d")
                        mn = tp.tile([P, csub, B], f32, tag="sp")
                        sp = tp.tile([P, csub, B], f32, tag="sq")
                        nc.vector.tensor_tensor(m[:], a, b, op=Alu.max)
                        nc.vector.tensor_tensor(mn[:], a, b, op=Alu.min)
                        nc.vector.tensor_tensor(mn[:], mn[:], m[:], op=Alu.subtract)
                        nc.scalar.activation(mn[:], mn[:], Act.Exp)
                        nc.scalar.activation(sp[:], mn[:], Act.Ln, bias=1.0)
                        nc.vector.tensor_add(h[:], m[:], sp[:])
                    # chunk produces rows [g_off, g_off + P*csub):
                    # row = g_off + p*csub + cc
                    nc.sync.dma_start(
                        dst_tile[g_off : g_off + P * csub, :].rearrange(
                            "(p c) b -> p (c b)", p=P
                        ),
                        h[:].rearrange("p c b -> p (c b)"),
                    )
                    g_off += P * csub
                    e_off += ch
    nc.compile()
    return nc


def host_prep(x, ptrs_list, seg_list):
    """Host-side sharding + pruning + index preprocessing -> per-core maps."""
    x = np.asarray(x, dtype=np.float32)
    for l, (n_out, f) in enumerate(zip(OUT_SIZES, FANINS)):
        seg = np.asarray(seg_list[l]).astype(np.int64)
        expected = np.repeat(np.arange(n_out, dtype=np.int64), f)
        assert np.array_equal(seg, expected), f"layer {l}: non-uniform segments"

    stages = plan(ptrs_list)
    idx_maps = {f"idx{l}": reorder_wrap(s) for l, s in enumerate(stages)}

    batch = x.shape[1]
    bpc = batch // NCORES
    in_maps = []
    for i in range(NCORES):
        xs = x[:, i * bpc : (i + 1) * bpc]
        # partition p, slot s holds var s*128+p (slot-major var layout)
        xv = np.ascontiguousarray(
            xs.reshape(S_ENC, P, bpc).transpose(1, 0, 2)
        ).reshape(P, -1)
        in_maps.append({"x": xv, **idx_maps})
    return in_maps


def _meta(stages):
    return tuple(
        (s["n_groups"], s["n_src_rows"], tuple(s["chunks"]), bool(s["direct"]))
        for s in stages
    )


_CACHE = {}


def _get_nc(meta=None):
    if meta is None:
        meta = _CACHE.get("meta")
        assert meta is not None, "call kernel() first"
    if _CACHE.get("meta") != meta:
        _CACHE["nc"] = build_nc(meta)
        _CACHE["meta"] = meta
    return _CACHE["nc"]


def kernel(x, ptrs0, seg0, ptrs1, seg1, ptrs2, seg2, ptrs3, seg3):
    from concourse.bass_utils import run_bass_kernel_spmd

    ptrs_list = [ptrs0, ptrs1, ptrs2, ptrs3]
    stages = plan(ptrs_list)
    resolve_direct(stages, x)
    nc = _get_nc(_meta(stages))
    in_maps = host_prep(x, ptrs_list, [seg0, seg1, seg2, seg3])
    res = run_bass_kernel_spmd(nc, in_maps, core_ids=list(range(NCORES)))
    outs = [r["out"] for r in res.results]
    full = np.concatenate(outs, axis=1)
    # rows were produced in readiness order; map back to natural order
    return np.ascontiguousarray(full[stages[1]["out_perm"]])


# revision 24
# speedup vs baseline: 2.0112x; 1.0092x over previous
"""Trainium2 Bass kernel for the sum-product "knowledge layer" network.

Computation (see problem reference):
  h0 = encode(x): 8194-row table [-inf, 0, pos0, neg0, pos1, neg1, ...]
       with pos = x (log-probs), neg = log(1 - exp(x)), per batch column.
  4 alternating layers, each: gather rows by ptrs, then segment-reduce over
  contiguous fanin groups (fanin 4 sum-of-logs "product" layers, fanin 2
  logsumexp "sum" layers).

Strategy (pure batch data-parallelism, 8 NeuronCores):
  - Shard the 512 batch columns 8 ways -> 64 columns per core.
  - Per core every tensor lives in DRAM as [rows, 64] fp32; one row = 256B.
  - Gathers use the SWDGE dma_gather instruction: int16 index list in SBUF,
    each index pulls one 256B row from the DRAM table; index list position j
    lands at SBUF partition j%128, free slot j//128.
  - DAG pruning (host side, per input set): working back from the 2048
    output rows, only rows actually referenced downstream are computed.
  - Layer fusion: layers 0+1 fuse into stage A, layers 2+3 into stage B.
    A stage group gathers its 8 leaf rows (2 product groups x fanin 4),
    sums each quad on DVE, then logsumexps the pair - so the intermediate
    product table never exists in DRAM, removing its store and a pipeline
    boundary, at the cost of recomputing product rows referenced by more
    than one sum edge (~4% more gather descriptors).
  - Cross-layer software pipelining: stage A's output rows are produced in
    chunk order (chunk ci stores rows [base, base + P*csub), row = base +
    p*csub + cc), A groups are sorted by the max table-0 row they
    reference, and every gather chunk's source AP is narrowed to the exact
    table prefix it needs, so the tile framework only serializes a gather
    against the stores that overlap its prefix. The encode is likewise
    chunked (vars are laid out slot-major: var v lives at partition v%128,
    slot v//128, so encode chunk j fills a table-0 row prefix). Chunk
    sizes ramp up then down so desc-gen and compute tails stay short.
  - Sum reduction: lse(a,b) = ln(e^a + e^b) computed directly (1 DVE add;
    Exp/Ln on the lightly-loaded ACT engine) - resolve_direct() verifies
    on the host, from the actual x values, that every e^arg stays a
    normal f32, falling back to max + ln(1+exp(min-max)) otherwise.  The
    Exp+Ln activation table is preloaded once (set 6) so the compiler
    inserts no per-instruction table reloads.
"""

import numpy as np

P = 128
B = 64  # batch columns per core
NCORES = 8
N_VARS = 4096
BATCH = 512
TAB0 = 2 * N_VARS + 2  # 8194
OUT_SIZES = [16384, 8192, 4096, 2048]
FANINS = [4, 2, 4, 2]
FE = 8  # edges per fused group: 2 (sum fanin) x 4 (product fanin)
CHUNK = 9216  # max gather indices per dma_gather instruction
S_ENC = N_VARS // P  # 32 encode slots per partition
ENC_CHUNKS = 4
SE = S_ENC // ENC_CHUNKS  # slots per encode chunk


def _pad_groups(n):
    return -(-n // P) * P


def _chunk_group_counts(n_groups):
    """Groups per dma_gather chunk (FE edges per group). Sizes ramp up at
    the start (small first chunk -> quick desc-gen once the source prefix
    lands) and down at the end (short compute tail -> the last store lands
    early and the consumer unblocks sooner)."""
    g = P
    rem = n_groups
    tail = []
    for s in (128, 128, 128, 256, 512):
        if rem >= s + g:
            tail.append(s)
            rem -= s
    head = []
    for s in (128, 256, 512):
        if rem >= s + g:
            head.append(s)
            rem -= s
    mid = []
    per = CHUNK // FE
    while rem > 0:
        s = min(per, rem)
        mid.append(s)
        rem -= s
    return head + mid + tail[::-1]


def plan(ptrs_list):
    """Prune the DAG bottom-up, fuse layer pairs, readiness-order stage A.

    Returns (stageA, stageB) dicts:
      n_groups: padded group count (stage A: pruned sum-layer-1 groups =
                rows of the intermediate table tA; stage B: 2048 outputs)
      n_src_rows: rows of the gathered table (A: TAB0, B: nA)
      chunks: list of (n_groups_in_chunk, src_prefix_rows)
      edge_src: per-edge source rows, production order, FE per group
    """
    p0, p1, p2, p3 = [np.asarray(p).astype(np.int64) for p in ptrs_list]
    # stage B: out group g needs L2 groups p3[2g], p3[2g+1]; each L2 group
    # h needs t-A rows p2[4h+k].
    b_l2 = p3.reshape(-1, 2)  # [2048, 2] L2 group ids
    b_src1 = p2.reshape(-1, 4)[b_l2]  # [2048, 2, 4] L1 (tA) compact... raw ids
    used1 = np.unique(b_src1)
    # stage A: one group per used L1 row; L1 row u needs L0 groups
    # p1[2u+j]; L0 group w needs t0 rows p0[4w+k].
    a_l0 = p1.reshape(-1, 2)[used1]  # [n1, 2] L0 group ids
    a_src0 = p0.reshape(-1, 4)[a_l0]  # [n1, 2, 4] t0 rows

    n1 = used1.size
    nA = _pad_groups(n1)
    srcA = np.zeros((nA, FE), dtype=np.int64)
    srcA[:n1] = a_src0.reshape(n1, FE)
    # readiness of a t0 row: encode chunk order (var slot v//128; rows 0/1
    # ready first)
    ready = np.maximum(srcA - 2, 0) // 2 // P
    rmax = ready.max(axis=1)
    # padding groups read only row 0: ready before any encode chunk, so
    # putting them first lets chunk 0's desc-gen+gather warm up under the
    # encode instead of idling the DMA engines.
    rmax[n1:] = -1
    order = np.argsort(rmax, kind="stable")
    srcA = srcA[order]
    prod = np.empty(nA, dtype=np.int64)
    prod[order] = np.arange(nA)  # compact A-group id -> production row

    relabel1 = prod[np.searchsorted(used1, b_src1)]  # [2048, 2, 4] tA rows
    srcB = relabel1.reshape(-1, FE)
    # Stage B output rows need no fixed order either - the host unpermutes
    # rows after the run - so readiness-sort B too: its early chunks then
    # only need a tA prefix and can overlap stage A's tail.
    orderB = np.argsort(srcB.max(axis=1), kind="stable")
    srcB = srcB[orderB]
    prodB = np.empty(srcB.shape[0], dtype=np.int64)
    prodB[orderB] = np.arange(srcB.shape[0])  # out group g -> produced row

    def mk(src, n_src_rows):
        n_groups = src.shape[0]
        chunks = []
        g_off = 0
        for gc in _chunk_group_counts(n_groups):
            m = int(src[g_off : g_off + gc].max()) + 1
            chunks.append((gc, m))
            g_off += gc
        return {
            "n_groups": n_groups,
            "n_src_rows": n_src_rows,
            "chunks": chunks,
            "edge_src": src.ravel(),
        }

    stages = [mk(srcA, TAB0), mk(srcB, nA)]
    stages[1]["out_perm"] = prodB

    # Logsumexp form per stage: lse(a,b) = ln(e^a + e^b) directly costs
    # 1 DVE op (vs 4) + whole-tile Exp on the lightly-loaded ACT engine,
    # but is only safe when e^min stays a normal f32.  resolve_direct()
    # checks the exact values once x is known.
    stages[0]["direct"] = None
    stages[1]["direct"] = None
    stages[0]["_srcB_quads"] = srcB.reshape(-1, 2, 4)
    return stages


def resolve_direct(stages, x):
    """Exact host-side bound check for the stage-B direct-form lse."""
    x = np.asarray(x, dtype=np.float64)
    pos = x
    neg = np.log1p(-np.exp(x))
    t0 = np.empty((TAB0, x.shape[1]))
    t0[0] = 0.0
    t0[1] = 0.0
    t0[2::2] = pos
    t0[3::2] = neg
    srcA = stages[0]["edge_src"].reshape(-1, 2, 4)
    q = t0[srcA].sum(axis=2)  # [nA, 2, cols]
    m = q.max(axis=1)
    mn = q.min(axis=1)
    tA = m + np.log1p(np.exp(mn - m))
    tb = tA[stages[0]["_srcB_quads"]].sum(axis=2)  # [2048, 2, cols]
    safe_a = q.min() > -80.0
    safe_b = tb.min() > -80.0
    stages[0]["direct"] = bool(safe_a)
    stages[1]["direct"] = bool(safe_b)


def reorder_wrap(stage):
    """Permute per-edge source ids into dma_gather order and wrap into the
    int16 [128, n_edges//16] SBUF layout (position j -> [j%16, j//16],
    replicated across the 8 gpsimd cores' 16-partition groups).

    Edge position j of chunk ci maps to partition p=j%128, slot=j//128,
    cc=slot//FE, k=slot%FE, production row = base_ci + p*csub + cc."""
    src = stage["edge_src"]
    out = np.empty(stage["n_groups"] * FE, dtype=np.int64)
    base = 0
    e_off = 0
    for gc, _m in stage["chunks"]:
        csub = gc // P
        n_e = gc * FE
        j = np.arange(n_e)
        p = j % P
        slot = j // P
        cc = slot // FE
        k = slot % FE
        row = base + p * csub + cc
        out[e_off : e_off + n_e] = src[row * FE + k]
        base += gc
        e_off += n_e
    assert out.max() < 2**15 and out.min() >= 0
    out = out.astype(np.int16)
    return np.ascontiguousarray(np.tile(out.reshape(-1, 16).T, (8, 1)))


def build_nc(meta):
    """meta: per-stage (n_groups, n_src_rows, chunks-tuple)."""
    import concourse.bacc as bacc
    import concourse.mybir as mybir
    import concourse.tile as tile

    f32 = mybir.dt.float32
    i16 = mybir.dt.int16
    Alu = mybir.AluOpType
    Act = mybir.ActivationFunctionType

    specs = [
        {"n_groups": n, "n_src_rows": s, "chunks": ch, "direct": dr}
        for (n, s, ch, dr) in meta
    ]

    nc = bacc.Bacc("TRN2", target_bir_lowering=False, debug=False)
    x = nc.dram_tensor("x", [P, S_ENC * B], f32, kind="ExternalInput")
    idx_in = [
        nc.dram_tensor(
            f"idx{l}", [P, FE * s["n_groups"] // 16], i16, kind="ExternalInput"
        )
        for l, s in enumerate(specs)
    ]
    out = nc.dram_tensor("out", [OUT_SIZES[3], B], f32, kind="ExternalOutput")

    with tile.TileContext(nc) as tc:
        with (
            tc.tile_pool(name="dram", bufs=1, space="DRAM") as dpool,
            tc.tile_pool(name="sb", bufs=4) as gp,
            tc.tile_pool(name="enc", bufs=ENC_CHUNKS) as ep,
            tc.tile_pool(name="hb", bufs=4) as hp,
            tc.tile_pool(name="tmp", bufs=2) as tp,
            tc.tile_pool(name="ix", bufs=1) as ixp,
        ):
            tables = [
                dpool.tile([s["n_src_rows"], B], f32, name=f"t{l}", tag=f"t{l}")
                for l, s in enumerate(specs)
            ]

            # Preload the combined Exp+Ln activation table once; the
            # insert_act_table_loads pass then finds every Exp/Ln already
            # covered and inserts no per-instruction reloads (1283ns each).
            ACT_SET_LN_EXP = 6  # natural_log_exp_and_others
            nc.scalar.add_instruction(
                mybir.InstLoadActFuncSet(
                    name=nc.get_next_instruction_name(),
                    ins=[],
                    outs=[],
                    act_func_set_id=ACT_SET_LN_EXP,
                )
            )

            # table0 rows 0 (-inf in the reference, never gathered) and 1
            # (zeros). Store first so the row prefix [0,2) is ready.
            z = ixp.tile([2, B], f32, tag="z")
            nc.vector.memset(z[:], 0.0)
            nc.sync.dma_start(tables[0][:][0:2, :], z[:])

            # --- encode, chunked: var v sits at partition v%128, slot
            # v//128; pos row 2+2v, neg row 3+2v.  Chunk j covers slots
            # [j*SE,(j+1)*SE) = rows [2+256*j*SE, 2+256*(j+1)*SE), a row
            # prefix, so stage-A gather chunks can start before the whole
            # encode finishes.
            # All independent loads are emitted before any store so the
            # in-order DMA queue never has a compute-gated store blocking a
            # ready load: x chunks first, then the index lists.
            xv = x[:].rearrange("p (s b) -> p s b", b=B)
            xls = []
            for j in range(ENC_CHUNKS):
                # contiguous destination: 2KB DMA descriptors (the
                # interleaved iv[:, :, 0, :] view would halve them to 256B
                # and pay the sub-512B 2x transfer penalty)
                xl = ep.tile([P, SE, B], f32, tag="xl")
                nc.sync.dma_start(xl[:], xv[:, j * SE : (j + 1) * SE, :])
                xls.append(xl)
            ix_t = []
            for l, s in enumerate(specs):
                t = ixp.tile([P, FE * s["n_groups"] // 16], i16, tag=f"ix{l}")
                nc.sync.dma_start(t[:], idx_in[l][:])
                ix_t.append(t)
            for j in range(ENC_CHUNKS):
                xl = xls[j]
                iv = ep.tile([P, SE, 2, B], f32, tag="enc")
                et = hp.tile([P, SE, B], f32, tag="h")
                nc.scalar.copy(iv[:][:, :, 0, :], xl[:])
                nc.scalar.activation(et[:], xl[:], Act.Exp)
                nc.scalar.activation(
                    iv[:][:, :, 1, :], et[:], Act.Ln, scale=-1.0, bias=1.0
                )
                r0 = 2 + 2 * P * SE * j
                r1 = 2 + 2 * P * SE * (j + 1)
                # row = r0 + 256*s + 2*p + k
                nc.sync.dma_start(
                    tables[0][:][r0:r1, :].rearrange("(s p k) b -> p s k b", p=P, k=2),
                    iv[:],
                )

            # --- fused gather + product-sum + logsumexp stages ---
            for l, s in enumerate(specs):
                dst_tile = tables[l + 1][:] if l + 1 < len(specs) else out[:]
                g_off = 0
                e_off = 0
                for gc, m_src in s["chunks"]:
                    csub = gc // P
                    ch = gc * FE
                    S = ch // P
                    g = gp.tile([P, S, B], f32, tag="g")
                    nc.gpsimd.dma_gather(
                        g[:],
                        tables[l][:][0:m_src, :],
                        ix_t[l][:, e_off // 16 : (e_off + ch) // 16],
                        ch,
                        ch,
                        B,
                        single_packet=False,
                    )
                    # [p, group, pair(2), fanin(4), b]
                    v = g[:].rearrange("p (c j k) b -> p c j k b", j=2, k=4)
                    s01 = tp.tile([P, csub, 2, B], f32, tag="m")
                    s23 = tp.tile([P, csub, 2, B], f32, tag="n")
                    ss = gp.tile([P, csub, 2, B], f32, tag="s")
                    nc.vector.tensor_add(s01[:], v[:, :, :, 0, :], v[:, :, :, 1, :])
                    nc.vector.tensor_add(s23[:], v[:, :, :, 2, :], v[:, :, :, 3, :])
                    nc.vector.tensor_add(ss[:], s01[:], s23[:])
                    a = ss[:][:, :, 0, :]
                    b = ss[:][:, :, 1, :]
                    h = hp.tile([P, csub, B], f32, tag="h")
                    if s["direct"]:
                        # lse(a,b) = ln(e^a + e^b): host verified e^min is a
                        # normal f32 (no scaling needed). 1 DVE op; the
                        # whole-tile Exp and the Ln ride the ACT engine.
                        e = tp.tile([P, csub, 2, B], f32, tag="d")
                        d = tp.tile([P, csub, B], f32, tag="sp")
                        nc.scalar.activation(e[:], ss[:], Act.Exp)
                        nc.vector.tensor_add(
                            d[:], e[:][:, :, 0, :], e[:][:, :, 1, :]
                        )
                        nc.scalar.activation(h[:], d[:], Act.Ln)
                    else:
                        # wider range: logsumexp = max + ln(1+exp(min-max))
                        m = tp.tile([P, csub, B], f32, tag="d")
                        mn = tp.tile([P, csub, B], f32, tag="sp")
                        sp = tp.tile([P, csub, B], f32, tag="sq")
                        nc.vector.tensor_tensor(m[:], a, b, op=Alu.max)
                        nc.vector.tensor_tensor(mn[:], a, b, op=Alu.min)
                        nc.vector.tensor_tensor(mn[:], mn[:], m[:], op=Alu.subtract)
                        nc.scalar.activation(mn[:], mn[:], Act.Exp)
                        nc.scalar.activation(sp[:], mn[:], Act.Ln, bias=1.0)
                        nc.vector.tensor_add(h[:], m[:], sp[:])
                    # chunk produces rows [g_off, g_off + P*csub):
                    # row = g_off + p*csub + cc
                    nc.sync.dma_start(
                        dst_tile[g_off : g_off + P * csub, :].rearrange(
                            "(p c) b -> p (c b)", p=P
                        ),
                        h[:].rearrange("p c b -> p (c b)"),
                    )
                    g_off += P * csub
                    e_off += ch
    nc.compile()
    return nc


def host_prep(x, ptrs_list, seg_list):
    """Host-side sharding + pruning + index preprocessing -> per-core maps."""
    x = np.asarray(x, dtype=np.float32)
    for l, (n_out, f) in enumerate(zip(OUT_SIZES, FANINS)):
        seg = np.asarray(seg_list[l]).astype(np.int64)
        expected = np.repeat(np.arange(n_out, dtype=np.int64), f)
        assert np.array_equal(seg, expected), f"layer {l}: non-uniform segments"

    stages = plan(ptrs_list)
    idx_maps = {f"idx{l}": reorder_wrap(s) for l, s in enumerate(stages)}

    batch = x.shape[1]
    bpc = batch // NCORES
    in_maps = []
    for i in range(NCORES):
        xs = x[:, i * bpc : (i + 1) * bpc]
        # partition p, slot s holds var s*128+p (slot-major var layout)
        xv = np.ascontiguousarray(
            xs.reshape(S_ENC, P, bpc).transpose(1, 0, 2)
        ).reshape(P, -1)
        in_maps.append({"x": xv, **idx_maps})
    return in_maps


def _meta(stages):
    return tuple(
        (s["n_groups"], s["n_src_rows"], tuple(s["chunks"]), bool(s["direct"]))
        for s in stages
    )


_CACHE = {}


def _get_nc(meta=None):
    if meta is None:
        meta = _CACHE.get("meta")
        assert meta is not None, "call kernel() first"
    if _CACHE.get("meta") != meta:
        _CACHE["nc"] = build_nc(meta)
        _CACHE["meta"] = meta
    return _CACHE["nc"]


def kernel(x, ptrs0, seg0, ptrs1, seg1, ptrs2, seg2, ptrs3, seg3):
    from concourse.bass_utils import run_bass_kernel_spmd

    ptrs_list = [ptrs0, ptrs1, ptrs2, ptrs3]
    stages = plan(ptrs_list)
    resolve_direct(stages, x)
    nc = _get_nc(_meta(stages))
    in_maps = host_prep(x, ptrs_list, [seg0, seg1, seg2, seg3])
    res = run_bass_kernel_spmd(nc, in_maps, core_ids=list(range(NCORES)))
    outs = [r["out"] for r in res.results]
    full = np.concatenate(outs, axis=1)
    # rows were produced in readiness order; map back to natural order
    return np.ascontiguousarray(full[stages[1]["out_perm"]])
